# revision 4
# baseline (speedup 1.0000x reference)
"""SOM update step on 8 Trainium2 NeuronCores — passthrough + sparse fixup.

Two collective-free launches (the baseline's mid-kernel AllGather cost
~1.05ms of barrier/transport time; see kernel_twopass_dense.py for the
previous dense two-pass version, sim ~42us):

  L1p (heavy, memory-roofline): stream (som|rv) bf16 once per core.
    Per tile, overlapped under the DMA stream: um9 BMU partial reduction
    (d1 = som - x, Square-with-accumulate), plus the full passthrough
    output (clip(som,0,1) | rv) -> out_t.  For every unit outside the BMU
    radius the reference update is exactly the identity (fm=0 -> som_new =
    clip(som); va=1 -> var_new = rv), so this IS the correct output there.
    Total traffic 6.4MB/core = the single-pass memory roofline.
  Host glue (O(units)): argmin over the 8x1152 device-computed distances,
    neighborhood factor maps in numpy f32 (using the real
    cartesian_distances input), affected-unit list (mask > 0, <=441 for
    r=10), gather those (som|rv) rows densely per core.
  L2s (tiny): one [128, 1568] tile per core, per-partition(=per-unit)
    factors, fused update, write back; host overlays the <=441 corrected
    rows over L1p's passthrough.  If the radius ever covers more than the
    1024-row capacity, fall back to the dense update launch (L2d) over
    all units.

bf16 I/O halves HBM traffic; validated against the 2e-2 rel-err gate at
1.13e-2 (worst case, in-disk units), deterministic.  The BMU argmin gap
(~2.1) is ~7x the worst-case bf16 perturbation of any unit distance.
"""
import numpy as np
import ml_dtypes

import concourse.bacc as bacc
import concourse.tile as tile
import concourse.bass_utils as bass_utils
from concourse import mybir

IMG = 28
N = 96
S = IMG * N            # 2688
NCORES = 8
UNITS = N * N          # 9216
UPC = UNITS // NCORES  # 1152 units per core
P = 128                # SBUF partitions
NT = UPC // P          # 9 tiles per core
B = IMG * IMG          # 784 block elements
FIX_CAP = P * NCORES   # 1024 sparse-fixup rows

F32 = mybir.dt.float32
BF16 = mybir.dt.bfloat16
OP = mybir.AluOpType
AF = mybir.ActivationFunctionType
NPBF = ml_dtypes.bfloat16

EPS_LOG = np.float32(1e-8)
RV_ALPHA = np.float32(0.9)


def _build_l1p(with_isr):
    """Heavy pass: um9 partials + passthrough output, one (som|rv) stream.

    with_isr=False (uniform rv): um = sum(d1^2)  (argmin is scale-invariant)
    with_isr=True  (general rv): um = sum((d1*isr)^2), isr = 1/sqrt(rv)
    shipped as a third block in the input row."""
    nb = 3 if with_isr else 2
    nc = bacc.Bacc("TRN2", num_devices=NCORES, debug=False)
    sv_d = nc.dram_tensor("sv", [UPC, nb * B], BF16, kind="ExternalInput")
    xb_d = nc.dram_tensor("xb", [P, B], BF16, kind="ExternalInput")
    out_d = nc.dram_tensor("out_t", [UPC, 2 * B], BF16, kind="ExternalOutput")
    um_d = nc.dram_tensor("um", [P, NT], F32, kind="ExternalOutput")

    with tile.TileContext(nc) as tc:
        with (
            tc.tile_pool(name="io", bufs=6) as io,
            tc.tile_pool(name="wk", bufs=4) as wk,
            tc.tile_pool(name="outp", bufs=4) as outp,
            tc.tile_pool(name="sm", bufs=1) as sm,
        ):
            xb = sm.tile([P, B], BF16, tag="xb")
            nc.scalar.dma_start(out=xb[:], in_=xb_d[:, :])
            um9 = sm.tile([P, NT], F32, tag="um9")
            for T in range(NT):
                r0 = P * T
                ti = io.tile([P, nb * B], BF16, tag="sv")
                nc.sync.dma_start(out=ti[:], in_=sv_d[r0:r0 + P, :])
                s_t, v_t = ti[:, 0:B], ti[:, B:2 * B]
                ot = outp.tile([P, 2 * B], BF16, tag="ot")
                nc.vector.tensor_scalar(out=ot[:, 0:B], in0=s_t,
                                        scalar1=0.0, scalar2=1.0,
                                        op0=OP.max, op1=OP.min)
                nc.vector.tensor_copy(ot[:, B:2 * B], v_t)
                d1 = wk.tile([P, B], BF16, tag="d1")
                nc.vector.tensor_tensor(d1[:], s_t, xb[:], OP.subtract)
                if with_isr:
                    t2 = wk.tile([P, B], BF16, tag="t2")
                    nc.vector.tensor_tensor(t2[:], d1[:], ti[:, 2 * B:3 * B],
                                            OP.mult)
                    d1 = t2
                scr = wk.tile([P, B], BF16, tag="scr")
                nc.scalar.activation(scr[:], d1[:], AF.Square,
                                     accum_out=um9[:, T:T + 1])
                nc.gpsimd.dma_start(out=out_d[r0:r0 + P, :], in_=ot[:])
            nc.sync.dma_start(out=um_d[:, :], in_=um9[:])
    nc.finalize()
    return nc


def _build_l2s():
    """Sparse fixup: one [128, 1568] tile = 128 affected unit rows with
    per-partition (per-unit) factors fn = -fm, va, sg = sqrt((1-va)(1-fm)^2):
      som_new = (d1 * fn) + som;  var_new = Square(d1 * sg) + va * rv."""
    nc = bacc.Bacc("TRN2", num_devices=NCORES, debug=False)
    svx_d = nc.dram_tensor("svx", [P, 2 * B], BF16, kind="ExternalInput")
    xb_d = nc.dram_tensor("xb", [P, B], BF16, kind="ExternalInput")
    fvg_d = nc.dram_tensor("fvg", [P, 3], F32, kind="ExternalInput")
    outx_d = nc.dram_tensor("outx", [P, 2 * B], BF16, kind="ExternalOutput")

    with tile.TileContext(nc) as tc:
        with (
            tc.tile_pool(name="sm", bufs=1) as sm,
        ):
            xb = sm.tile([P, B], BF16, tag="xb")
            nc.scalar.dma_start(out=xb[:], in_=xb_d[:, :])
            fvg = sm.tile([P, 3], F32, tag="fvg")
            nc.scalar.dma_start(out=fvg[:], in_=fvg_d[:, :])
            ti = sm.tile([P, 2 * B], BF16, tag="svx")
            nc.sync.dma_start(out=ti[:], in_=svx_d[:, :])
            s_t, v_t = ti[:, 0:B], ti[:, B:2 * B]
            av = sm.tile([P, B], BF16, tag="av")
            nc.vector.tensor_scalar(out=av[:], in0=v_t, scalar1=fvg[:, 1:2],
                                    scalar2=None, op0=OP.mult)
            d1 = sm.tile([P, B], BF16, tag="d1")
            nc.vector.tensor_tensor(d1[:], s_t, xb[:], OP.subtract)
            q2 = sm.tile([P, B], BF16, tag="q2")
            nc.scalar.activation(q2[:], d1[:], AF.Square, scale=fvg[:, 2:3])
            m1 = sm.tile([P, B], BF16, tag="m1")
            nc.vector.tensor_scalar(out=m1[:], in0=d1[:], scalar1=fvg[:, 0:1],
                                    scalar2=None, op0=OP.mult)
            ot = sm.tile([P, 2 * B], BF16, tag="ot")
            nc.vector.tensor_tensor(ot[:, 0:B], m1[:], s_t, OP.add)
            nc.gpsimd.dma_start(out=outx_d[:, 0:B], in_=ot[:, 0:B])
            nc.vector.tensor_tensor(ot[:, B:2 * B], q2[:], av[:], OP.add)
            nc.gpsimd.dma_start(out=outx_d[:, B:2 * B], in_=ot[:, B:2 * B])
    nc.finalize()
    return nc


def _build_l2d():
    """Dense fallback update over all units (per-tile-column factors)."""
    nc = bacc.Bacc("TRN2", num_devices=NCORES, debug=False)
    sv_d = nc.dram_tensor("sv", [UPC, 2 * B], BF16, kind="ExternalInput")
    xb_d = nc.dram_tensor("xb", [P, B], BF16, kind="ExternalInput")
    fvg_d = nc.dram_tensor("fvg", [P, 3 * NT], F32, kind="ExternalInput")
    out_d = nc.dram_tensor("out_t", [UPC, 2 * B], BF16, kind="ExternalOutput")

    with tile.TileContext(nc) as tc:
        with (
            tc.tile_pool(name="io", bufs=6) as io,
            tc.tile_pool(name="wk", bufs=4) as wk,
            tc.tile_pool(name="outp", bufs=4) as outp,
            tc.tile_pool(name="sm", bufs=1) as sm,
        ):
            xb = sm.tile([P, B], BF16, tag="xb")
            nc.scalar.dma_start(out=xb[:], in_=xb_d[:, :])
            fvg = sm.tile([P, 3 * NT], F32, tag="fvg")
            nc.scalar.dma_start(out=fvg[:], in_=fvg_d[:, :])
            fn = fvg[:, 0:NT]
            va = fvg[:, NT:2 * NT]
            sg = fvg[:, 2 * NT:3 * NT]
            for T in range(NT):
                r0 = P * T
                ti = io.tile([P, 2 * B], BF16, tag="sv")
                nc.sync.dma_start(out=ti[:], in_=sv_d[r0:r0 + P, :])
                s_t, v_t = ti[:, 0:B], ti[:, B:2 * B]
                d1 = wk.tile([P, B], BF16, tag="d1")
                nc.vector.tensor_tensor(d1[:], s_t, xb[:], OP.subtract)
                m1 = wk.tile([P, B], BF16, tag="m1")
                nc.scalar.mul(m1[:], d1[:], fn[:, T:T + 1])
                q2 = wk.tile([P, B], BF16, tag="q2")
                nc.scalar.activation(q2[:], d1[:], AF.Square,
                                     scale=sg[:, T:T + 1])
                av = wk.tile([P, B], BF16, tag="av")
                nc.vector.tensor_scalar(out=av[:], in0=v_t,
                                        scalar1=va[:, T:T + 1], scalar2=None,
                                        op0=OP.mult)
                ot = outp.tile([P, 2 * B], BF16, tag="ot")
                nc.vector.tensor_tensor(ot[:, 0:B], m1[:], s_t, OP.add)
                nc.vector.tensor_tensor(ot[:, B:2 * B], q2[:], av[:], OP.add)
                nc.gpsimd.dma_start(out=out_d[r0:r0 + P, :], in_=ot[:])
    nc.finalize()
    return nc


_CACHE = {}


def _get(name, builder, *args):
    if name not in _CACHE:
        _CACHE[name] = builder(*args)
    return _CACHE[name]


def _unit_major(a):
    """[S, S] -> [9216, 784]: one 28x28 block per row."""
    return (np.ascontiguousarray(a).reshape(N, IMG, N, IMG)
            .transpose(0, 2, 1, 3).reshape(UNITS, B))


def kernel(som, running_variance, learning_rates, radius,
           cartesian_distances, x):
    som = np.asarray(som, np.float32)
    rv = np.asarray(running_variance, np.float32)
    lr = np.asarray(learning_rates, np.float32)
    rad = np.asarray(radius, np.float32)
    cd = np.asarray(cartesian_distances, np.float32)
    x = np.asarray(x, np.float32)

    som_t = _unit_major(som)
    rv_t = _unit_major(rv)
    som_b = som_t.astype(NPBF)
    rv_b = rv_t.astype(NPBF)
    xb = np.broadcast_to(x.reshape(1, B), (P, B)).astype(NPBF)
    xb = np.ascontiguousarray(xb)

    uniform = bool((rv == rv.flat[0]).all()) and rv.flat[0] > 0
    if uniform:
        sv_full = np.concatenate([som_b, rv_b], axis=1)
        nc1 = _get("l1p", _build_l1p, False)
    else:
        isr_b = (np.float32(1.0) / np.sqrt(rv_t)).astype(NPBF)
        sv_full = np.concatenate([som_b, rv_b, isr_b], axis=1)
        nc1 = _get("l1pg", _build_l1p, True)

    l1_maps = [{
        "sv": np.ascontiguousarray(sv_full[UPC * c:UPC * (c + 1)]),
        "xb": xb,
    } for c in range(NCORES)]
    res1 = bass_utils.run_bass_kernel_spmd(
        nc1, l1_maps, core_ids=list(range(NCORES)))

    # passthrough output (exact for every out-of-radius unit) + um partials
    out_t = np.concatenate(
        [np.asarray(res1.results[c]["out_t"]) for c in range(NCORES)], axis=0)
    um = np.concatenate(
        [np.asarray(res1.results[c]["um"], np.float32).T.reshape(-1)
         for c in range(NCORES)])

    # ---- host glue: argmin + neighborhood factor maps (numpy f32) ----
    g = int(np.argmin(um))
    bi, bj = g // N, g % N
    r = rad[bi, bj]
    lr_s = lr[bi, bj]
    dist_mod = np.float32(1.0) / (np.float32(2.0) * r * r)
    constant = -np.log(EPS_LOG / lr_s) / dist_mod
    d = cd[:, :, bi, bj]
    mask = np.where(d > r, np.float32(0.0), np.float32(1.0))
    fm = mask * lr * np.exp(-d * dist_mod)
    va = RV_ALPHA - np.float32(0.5) + np.float32(1.0) / (
        np.float32(1.0) + np.exp(-d / constant))
    va = np.clip(va * mask + (np.float32(1.0) - mask),
                 np.float32(0.0), np.float32(1.0))
    fn_m = (-fm).reshape(-1).astype(np.float32)
    va_m = va.reshape(-1).astype(np.float32)
    sg_m = np.sqrt((np.float32(1.0) - va) * (np.float32(1.0) - fm) ** 2
                   ).reshape(-1).astype(np.float32)

    sv2 = sv_full[:, 0:2 * B]
    idx = np.flatnonzero(mask.reshape(-1) > 0)
    if idx.size == 0:
        pass  # empty neighborhood: passthrough IS the full update
    elif idx.size <= FIX_CAP:
        # ---- sparse fixup: <=1024 affected rows, 128 per core ----
        k = idx.size
        idx_pad = np.concatenate(
            [idx, np.full(FIX_CAP - k, idx[0], np.int64)])
        fvg = np.zeros((FIX_CAP, 3), np.float32)
        fvg[:k, 0] = fn_m[idx]
        fvg[:k, 1] = va_m[idx]
        fvg[:k, 2] = sg_m[idx]
        fvg[k:, 1] = 1.0        # padding rows: identity update
        l2_maps = [{
            "svx": np.ascontiguousarray(sv2[idx_pad[P * c:P * (c + 1)]]),
            "xb": xb,
            "fvg": np.ascontiguousarray(fvg[P * c:P * (c + 1)]),
        } for c in range(NCORES)]
        res2 = bass_utils.run_bass_kernel_spmd(
            _get("l2s", _build_l2s), l2_maps, core_ids=list(range(NCORES)))
        fix = np.concatenate(
            [np.asarray(res2.results[c]["outx"]) for c in range(NCORES)],
            axis=0)
        out_t[idx] = fix[:k]
    else:
        # ---- dense fallback: update every unit ----
        def shard(vec, c):
            return vec[UPC * c:UPC * (c + 1)].reshape(NT, P).T.copy()
        l2_maps = [{
            "sv": np.ascontiguousarray(sv2[UPC * c:UPC * (c + 1)]),
            "xb": xb,
            "fvg": np.ascontiguousarray(np.concatenate(
                [shard(fn_m, c), shard(va_m, c), shard(sg_m, c)], axis=1)),
        } for c in range(NCORES)]
        res2 = bass_utils.run_bass_kernel_spmd(
            _get("l2d", _build_l2d), l2_maps, core_ids=list(range(NCORES)))
        out_t = np.concatenate(
            [np.asarray(res2.results[c]["out_t"]) for c in range(NCORES)],
            axis=0)

    out_t = out_t.astype(np.float32)
    sn_t, vn_t = out_t[:, 0:B], out_t[:, B:2 * B]

    def untile(a):
        return (a.reshape(N, N, IMG, IMG).transpose(0, 2, 1, 3)
                .reshape(S, S))

    return np.stack([untile(sn_t), untile(vn_t)]).astype(np.float32)


# revision 5
# speedup vs baseline: 1.1089x; 1.1089x over previous
"""SOM update step on 8 Trainium2 NeuronCores — passthrough + sparse fixup.

Two collective-free launches (the baseline's mid-kernel AllGather cost
~1.05ms of barrier/transport time; see kernel_twopass_dense.py for the
previous dense two-pass version, sim ~42us):

  L1p (heavy, memory-roofline): stream (som|rv) bf16 once per core.
    Per tile, overlapped under the DMA stream: um9 BMU partial reduction
    (d1 = som - x, Square-with-accumulate), plus the full passthrough
    output (clip(som,0,1) | rv) -> out_t.  For every unit outside the BMU
    radius the reference update is exactly the identity (fm=0 -> som_new =
    clip(som); va=1 -> var_new = rv), so this IS the correct output there.
    Total traffic 6.4MB/core = the single-pass memory roofline.
  Host glue (O(units)): argmin over the 8x1152 device-computed distances,
    neighborhood factor maps in numpy f32 (using the real
    cartesian_distances input), affected-unit list (mask > 0, <=441 for
    r=10), gather those (som|rv) rows densely per core.
  L2s (tiny): one [128, 1568] tile per core, per-partition(=per-unit)
    factors, fused update, write back; host overlays the <=441 corrected
    rows over L1p's passthrough.  If the radius ever covers more than the
    1024-row capacity, fall back to the dense update launch (L2d) over
    all units.

bf16 I/O halves HBM traffic; validated against the 2e-2 rel-err gate at
1.13e-2 (worst case, in-disk units), deterministic.  The BMU argmin gap
(~2.1) is ~7x the worst-case bf16 perturbation of any unit distance.
"""
import numpy as np
import ml_dtypes

import concourse.bacc as bacc
import concourse.tile as tile
import concourse.bass_utils as bass_utils
from concourse import mybir

IMG = 28
N = 96
S = IMG * N            # 2688
NCORES = 8
UNITS = N * N          # 9216
UPC = UNITS // NCORES  # 1152 units per core
P = 128                # SBUF partitions
NT = UPC // P          # 9 tiles per core
B = IMG * IMG          # 784 block elements
FIX_CAP = P * NCORES   # 1024 sparse-fixup rows

F32 = mybir.dt.float32
BF16 = mybir.dt.bfloat16
OP = mybir.AluOpType
AF = mybir.ActivationFunctionType
NPBF = ml_dtypes.bfloat16

EPS_LOG = np.float32(1e-8)
RV_ALPHA = np.float32(0.9)


def _build_l1p(with_isr):
    """Heavy pass: um9 partials + passthrough output, one (som|rv) stream.

    with_isr=False (uniform rv): um = sum(d1^2)  (argmin is scale-invariant)
    with_isr=True  (general rv): um = sum((d1*isr)^2), isr = 1/sqrt(rv)
    shipped as a third block in the input row."""
    nb = 3 if with_isr else 2
    nc = bacc.Bacc("TRN2", num_devices=NCORES, debug=False)
    sv_d = nc.dram_tensor("sv", [UPC, nb * B], BF16, kind="ExternalInput")
    xb_d = nc.dram_tensor("xb", [P, B], BF16, kind="ExternalInput")
    out_d = nc.dram_tensor("out_t", [UPC, 2 * B], BF16, kind="ExternalOutput")
    um_d = nc.dram_tensor("um", [P, NT], F32, kind="ExternalOutput")

    with tile.TileContext(nc) as tc:
        with (
            tc.tile_pool(name="io", bufs=6) as io,
            tc.tile_pool(name="wk", bufs=4) as wk,
            tc.tile_pool(name="outp", bufs=4) as outp,
            tc.tile_pool(name="sm", bufs=1) as sm,
        ):
            xb = sm.tile([P, B], BF16, tag="xb")
            nc.scalar.dma_start(out=xb[:], in_=xb_d[:, :])
            um9 = sm.tile([P, NT], F32, tag="um9")
            for T in range(NT):
                r0 = P * T
                ti = io.tile([P, nb * B], BF16, tag="sv")
                nc.sync.dma_start(out=ti[:], in_=sv_d[r0:r0 + P, :])
                s_t, v_t = ti[:, 0:B], ti[:, B:2 * B]
                ot = outp.tile([P, 2 * B], BF16, tag="ot")
                nc.vector.tensor_scalar(out=ot[:, 0:B], in0=s_t,
                                        scalar1=0.0, scalar2=1.0,
                                        op0=OP.max, op1=OP.min)
                nc.vector.tensor_copy(ot[:, B:2 * B], v_t)
                d1 = wk.tile([P, B], BF16, tag="d1")
                nc.vector.tensor_tensor(d1[:], s_t, xb[:], OP.subtract)
                if with_isr:
                    t2 = wk.tile([P, B], BF16, tag="t2")
                    nc.vector.tensor_tensor(t2[:], d1[:], ti[:, 2 * B:3 * B],
                                            OP.mult)
                    d1 = t2
                scr = wk.tile([P, B], BF16, tag="scr")
                nc.scalar.activation(scr[:], d1[:], AF.Square,
                                     accum_out=um9[:, T:T + 1])
                nc.gpsimd.dma_start(out=out_d[r0:r0 + P, :], in_=ot[:])
            nc.sync.dma_start(out=um_d[:, :], in_=um9[:])
    nc.finalize()
    return nc


def _build_l1pu2():
    """Uniform-rv heavy pass: som-only input stream.  The var-plane
    passthrough is a constant fill (var_new = rv exactly for out-of-radius
    units, and rv is uniform), so instead of reading 1.6MB of rv just to
    write the same bytes back, the runtime scalar arrives as a [P,1] input
    and is broadcast on-device into one SBUF tile that feeds all nine
    var-plane writes.  Halves L1p input traffic: DMA busy 20.7 -> 15.7us.
    var-plane out-DMAs ride the scalar/HWDGE queue (gpsimd would serialize
    them against the som-plane out-DMAs on the SWDGE engine)."""
    nc = bacc.Bacc("TRN2", num_devices=NCORES, debug=False)
    s_d = nc.dram_tensor("s", [UPC, B], BF16, kind="ExternalInput")
    xb_d = nc.dram_tensor("xb", [P, B], BF16, kind="ExternalInput")
    rvv_d = nc.dram_tensor("rvv", [P, 1], F32, kind="ExternalInput")
    out_d = nc.dram_tensor("out_t", [UPC, 2 * B], BF16, kind="ExternalOutput")
    um_d = nc.dram_tensor("um", [P, NT], F32, kind="ExternalOutput")

    with tile.TileContext(nc) as tc:
        with (
            tc.tile_pool(name="io", bufs=6) as io,
            tc.tile_pool(name="wk", bufs=4) as wk,
            tc.tile_pool(name="outp", bufs=4) as outp,
            tc.tile_pool(name="sm", bufs=1) as sm,
        ):
            xb = sm.tile([P, B], BF16, tag="xb")
            nc.scalar.dma_start(out=xb[:], in_=xb_d[:, :])
            rvv = sm.tile([P, 1], F32, tag="rvv")
            nc.scalar.dma_start(out=rvv[:], in_=rvv_d[:, :])
            vhalf = sm.tile([P, B], BF16, tag="vhalf")
            nc.vector.tensor_scalar(out=vhalf[:], in0=xb[:], scalar1=0.0,
                                    scalar2=rvv[:], op0=OP.mult, op1=OP.add)
            um9 = sm.tile([P, NT], F32, tag="um9")
            for T in range(NT):
                r0 = P * T
                ti = io.tile([P, B], BF16, tag="s")
                nc.sync.dma_start(out=ti[:], in_=s_d[r0:r0 + P, :])
                ot = outp.tile([P, B], BF16, tag="ot")
                nc.vector.tensor_scalar(out=ot[:], in0=ti[:], scalar1=0.0,
                                        scalar2=1.0, op0=OP.max, op1=OP.min)
                d1 = wk.tile([P, B], BF16, tag="d1")
                nc.vector.tensor_tensor(d1[:], ti[:], xb[:], OP.subtract)
                scr = wk.tile([P, B], BF16, tag="scr")
                nc.scalar.activation(scr[:], d1[:], AF.Square,
                                     accum_out=um9[:, T:T + 1])
                nc.gpsimd.dma_start(out=out_d[r0:r0 + P, 0:B], in_=ot[:])
                nc.scalar.dma_start(out=out_d[r0:r0 + P, B:2 * B],
                                    in_=vhalf[:])
            nc.sync.dma_start(out=um_d[:, :], in_=um9[:])
    nc.finalize()
    return nc


def _build_l2s():
    """Sparse fixup: one [128, 1568] tile = 128 affected unit rows with
    per-partition (per-unit) factors fn = -fm, va, sg = sqrt((1-va)(1-fm)^2):
      som_new = (d1 * fn) + som;  var_new = Square(d1 * sg) + va * rv."""
    nc = bacc.Bacc("TRN2", num_devices=NCORES, debug=False)
    svx_d = nc.dram_tensor("svx", [P, 2 * B], BF16, kind="ExternalInput")
    xb_d = nc.dram_tensor("xb", [P, B], BF16, kind="ExternalInput")
    fvg_d = nc.dram_tensor("fvg", [P, 3], F32, kind="ExternalInput")
    outx_d = nc.dram_tensor("outx", [P, 2 * B], BF16, kind="ExternalOutput")

    with tile.TileContext(nc) as tc:
        with (
            tc.tile_pool(name="sm", bufs=1) as sm,
        ):
            xb = sm.tile([P, B], BF16, tag="xb")
            nc.scalar.dma_start(out=xb[:], in_=xb_d[:, :])
            fvg = sm.tile([P, 3], F32, tag="fvg")
            nc.scalar.dma_start(out=fvg[:], in_=fvg_d[:, :])
            ti = sm.tile([P, 2 * B], BF16, tag="svx")
            nc.sync.dma_start(out=ti[:], in_=svx_d[:, :])
            s_t, v_t = ti[:, 0:B], ti[:, B:2 * B]
            av = sm.tile([P, B], BF16, tag="av")
            nc.vector.tensor_scalar(out=av[:], in0=v_t, scalar1=fvg[:, 1:2],
                                    scalar2=None, op0=OP.mult)
            d1 = sm.tile([P, B], BF16, tag="d1")
            nc.vector.tensor_tensor(d1[:], s_t, xb[:], OP.subtract)
            q2 = sm.tile([P, B], BF16, tag="q2")
            nc.scalar.activation(q2[:], d1[:], AF.Square, scale=fvg[:, 2:3])
            m1 = sm.tile([P, B], BF16, tag="m1")
            nc.vector.tensor_scalar(out=m1[:], in0=d1[:], scalar1=fvg[:, 0:1],
                                    scalar2=None, op0=OP.mult)
            ot = sm.tile([P, 2 * B], BF16, tag="ot")
            nc.vector.tensor_tensor(ot[:, 0:B], m1[:], s_t, OP.add)
            nc.gpsimd.dma_start(out=outx_d[:, 0:B], in_=ot[:, 0:B])
            nc.vector.tensor_tensor(ot[:, B:2 * B], q2[:], av[:], OP.add)
            nc.gpsimd.dma_start(out=outx_d[:, B:2 * B], in_=ot[:, B:2 * B])
    nc.finalize()
    return nc


def _build_l2d():
    """Dense fallback update over all units (per-tile-column factors)."""
    nc = bacc.Bacc("TRN2", num_devices=NCORES, debug=False)
    sv_d = nc.dram_tensor("sv", [UPC, 2 * B], BF16, kind="ExternalInput")
    xb_d = nc.dram_tensor("xb", [P, B], BF16, kind="ExternalInput")
    fvg_d = nc.dram_tensor("fvg", [P, 3 * NT], F32, kind="ExternalInput")
    out_d = nc.dram_tensor("out_t", [UPC, 2 * B], BF16, kind="ExternalOutput")

    with tile.TileContext(nc) as tc:
        with (
            tc.tile_pool(name="io", bufs=6) as io,
            tc.tile_pool(name="wk", bufs=4) as wk,
            tc.tile_pool(name="outp", bufs=4) as outp,
            tc.tile_pool(name="sm", bufs=1) as sm,
        ):
            xb = sm.tile([P, B], BF16, tag="xb")
            nc.scalar.dma_start(out=xb[:], in_=xb_d[:, :])
            fvg = sm.tile([P, 3 * NT], F32, tag="fvg")
            nc.scalar.dma_start(out=fvg[:], in_=fvg_d[:, :])
            fn = fvg[:, 0:NT]
            va = fvg[:, NT:2 * NT]
            sg = fvg[:, 2 * NT:3 * NT]
            for T in range(NT):
                r0 = P * T
                ti = io.tile([P, 2 * B], BF16, tag="sv")
                nc.sync.dma_start(out=ti[:], in_=sv_d[r0:r0 + P, :])
                s_t, v_t = ti[:, 0:B], ti[:, B:2 * B]
                d1 = wk.tile([P, B], BF16, tag="d1")
                nc.vector.tensor_tensor(d1[:], s_t, xb[:], OP.subtract)
                m1 = wk.tile([P, B], BF16, tag="m1")
                nc.scalar.mul(m1[:], d1[:], fn[:, T:T + 1])
                q2 = wk.tile([P, B], BF16, tag="q2")
                nc.scalar.activation(q2[:], d1[:], AF.Square,
                                     scale=sg[:, T:T + 1])
                av = wk.tile([P, B], BF16, tag="av")
                nc.vector.tensor_scalar(out=av[:], in0=v_t,
                                        scalar1=va[:, T:T + 1], scalar2=None,
                                        op0=OP.mult)
                ot = outp.tile([P, 2 * B], BF16, tag="ot")
                nc.vector.tensor_tensor(ot[:, 0:B], m1[:], s_t, OP.add)
                nc.vector.tensor_tensor(ot[:, B:2 * B], q2[:], av[:], OP.add)
                nc.gpsimd.dma_start(out=out_d[r0:r0 + P, :], in_=ot[:])
    nc.finalize()
    return nc


_CACHE = {}


def _get(name, builder, *args):
    if name not in _CACHE:
        _CACHE[name] = builder(*args)
    return _CACHE[name]


def _unit_major(a):
    """[S, S] -> [9216, 784]: one 28x28 block per row."""
    return (np.ascontiguousarray(a).reshape(N, IMG, N, IMG)
            .transpose(0, 2, 1, 3).reshape(UNITS, B))


def kernel(som, running_variance, learning_rates, radius,
           cartesian_distances, x):
    som = np.asarray(som, np.float32)
    rv = np.asarray(running_variance, np.float32)
    lr = np.asarray(learning_rates, np.float32)
    rad = np.asarray(radius, np.float32)
    cd = np.asarray(cartesian_distances, np.float32)
    x = np.asarray(x, np.float32)

    som_t = _unit_major(som)
    som_b = som_t.astype(NPBF)
    xb = np.broadcast_to(x.reshape(1, B), (P, B)).astype(NPBF)
    xb = np.ascontiguousarray(xb)

    uniform = bool((rv == rv.flat[0]).all()) and rv.flat[0] > 0
    if uniform:
        rvv = np.full((P, 1), rv.flat[0], np.float32)
        l1_maps = [{
            "s": np.ascontiguousarray(som_b[UPC * c:UPC * (c + 1)]),
            "xb": xb,
            "rvv": rvv,
        } for c in range(NCORES)]
        res1 = bass_utils.run_bass_kernel_spmd(
            _get("l1pu2", _build_l1pu2), l1_maps,
            core_ids=list(range(NCORES)))

        def rv_rows(rows):
            return np.full((len(rows), B), rv.flat[0], NPBF)
    else:
        rv_t = _unit_major(rv)
        rv_b = rv_t.astype(NPBF)
        isr_b = (np.float32(1.0) / np.sqrt(rv_t)).astype(NPBF)
        sv_full = np.concatenate([som_b, rv_b, isr_b], axis=1)
        l1_maps = [{
            "sv": np.ascontiguousarray(sv_full[UPC * c:UPC * (c + 1)]),
            "xb": xb,
        } for c in range(NCORES)]
        res1 = bass_utils.run_bass_kernel_spmd(
            _get("l1pg", _build_l1p, True), l1_maps,
            core_ids=list(range(NCORES)))

        def rv_rows(rows):
            return rv_b[rows]

    # passthrough output (exact for every out-of-radius unit) + um partials
    out_t = np.concatenate(
        [np.asarray(res1.results[c]["out_t"]) for c in range(NCORES)], axis=0)
    um = np.concatenate(
        [np.asarray(res1.results[c]["um"], np.float32).T.reshape(-1)
         for c in range(NCORES)])

    # ---- host glue: argmin + neighborhood factor maps (numpy f32) ----
    g = int(np.argmin(um))
    bi, bj = g // N, g % N
    r = rad[bi, bj]
    lr_s = lr[bi, bj]
    dist_mod = np.float32(1.0) / (np.float32(2.0) * r * r)
    constant = -np.log(EPS_LOG / lr_s) / dist_mod
    d = cd[:, :, bi, bj]
    mask = np.where(d > r, np.float32(0.0), np.float32(1.0))
    fm = mask * lr * np.exp(-d * dist_mod)
    va = RV_ALPHA - np.float32(0.5) + np.float32(1.0) / (
        np.float32(1.0) + np.exp(-d / constant))
    va = np.clip(va * mask + (np.float32(1.0) - mask),
                 np.float32(0.0), np.float32(1.0))
    fn_m = (-fm).reshape(-1).astype(np.float32)
    va_m = va.reshape(-1).astype(np.float32)
    sg_m = np.sqrt((np.float32(1.0) - va) * (np.float32(1.0) - fm) ** 2
                   ).reshape(-1).astype(np.float32)

    idx = np.flatnonzero(mask.reshape(-1) > 0)
    if idx.size == 0:
        pass  # empty neighborhood: passthrough IS the full update
    elif idx.size <= FIX_CAP:
        # ---- sparse fixup: <=1024 affected rows, 128 per core ----
        k = idx.size
        idx_pad = np.concatenate(
            [idx, np.full(FIX_CAP - k, idx[0], np.int64)])
        svx = np.concatenate([som_b[idx_pad], rv_rows(idx_pad)], axis=1)
        fvg = np.zeros((FIX_CAP, 3), np.float32)
        fvg[:k, 0] = fn_m[idx]
        fvg[:k, 1] = va_m[idx]
        fvg[:k, 2] = sg_m[idx]
        fvg[k:, 1] = 1.0        # padding rows: identity update
        l2_maps = [{
            "svx": np.ascontiguousarray(svx[P * c:P * (c + 1)]),
            "xb": xb,
            "fvg": np.ascontiguousarray(fvg[P * c:P * (c + 1)]),
        } for c in range(NCORES)]
        res2 = bass_utils.run_bass_kernel_spmd(
            _get("l2s", _build_l2s), l2_maps, core_ids=list(range(NCORES)))
        fix = np.concatenate(
            [np.asarray(res2.results[c]["outx"]) for c in range(NCORES)],
            axis=0)
        out_t[idx] = fix[:k]
    else:
        # ---- dense fallback: update every unit ----
        def shard(vec, c):
            return vec[UPC * c:UPC * (c + 1)].reshape(NT, P).T.copy()
        all_rows = np.arange(UNITS)
        sv2 = np.concatenate([som_b, rv_rows(all_rows)], axis=1)
        l2_maps = [{
            "sv": np.ascontiguousarray(sv2[UPC * c:UPC * (c + 1)]),
            "xb": xb,
            "fvg": np.ascontiguousarray(np.concatenate(
                [shard(fn_m, c), shard(va_m, c), shard(sg_m, c)], axis=1)),
        } for c in range(NCORES)]
        res2 = bass_utils.run_bass_kernel_spmd(
            _get("l2d", _build_l2d), l2_maps, core_ids=list(range(NCORES)))
        out_t = np.concatenate(
            [np.asarray(res2.results[c]["out_t"]) for c in range(NCORES)],
            axis=0)

    out_t = out_t.astype(np.float32)
    sn_t, vn_t = out_t[:, 0:B], out_t[:, B:2 * B]

    def untile(a):
        return (a.reshape(N, N, IMG, IMG).transpose(0, 2, 1, 3)
                .reshape(S, S))

    return np.stack([untile(sn_t), untile(vn_t)]).astype(np.float32)


# revision 6
# speedup vs baseline: 1.1575x; 1.0437x over previous
"""SOM update step on 8 Trainium2 NeuronCores — passthrough + sparse fixup.

Two collective-free launches (the baseline's mid-kernel AllGather cost
~1.05ms of barrier/transport time; see kernel_twopass_dense.py for the
previous dense two-pass version, sim ~42us):

  L1p (heavy, memory-roofline): stream (som|rv) bf16 once per core.
    Per tile, overlapped under the DMA stream: um9 BMU partial reduction
    (d1 = som - x, Square-with-accumulate), plus the full passthrough
    output (clip(som,0,1) | rv) -> out_t.  For every unit outside the BMU
    radius the reference update is exactly the identity (fm=0 -> som_new =
    clip(som); va=1 -> var_new = rv), so this IS the correct output there.
    Total traffic 6.4MB/core = the single-pass memory roofline.
  Host glue (O(units)): argmin over the 8x1152 device-computed distances,
    neighborhood factor maps in numpy f32 (using the real
    cartesian_distances input), affected-unit list (mask > 0, <=441 for
    r=10), gather those (som|rv) rows densely per core.
  L2s (tiny): one [128, 1568] tile per core, per-partition(=per-unit)
    factors, fused update, write back; host overlays the <=441 corrected
    rows over L1p's passthrough.  If the radius ever covers more than the
    1024-row capacity, fall back to the dense update launch (L2d) over
    all units.

bf16 I/O halves HBM traffic; validated against the 2e-2 rel-err gate at
1.13e-2 (worst case, in-disk units), deterministic.  The BMU argmin gap
(~2.1) is ~7x the worst-case bf16 perturbation of any unit distance.
"""
import numpy as np
import ml_dtypes

import concourse.bacc as bacc
import concourse.tile as tile
import concourse.bass_utils as bass_utils
from concourse import mybir

IMG = 28
N = 96
S = IMG * N            # 2688
NCORES = 8
UNITS = N * N          # 9216
UPC = UNITS // NCORES  # 1152 units per core
P = 128                # SBUF partitions
NT = UPC // P          # 9 tiles per core
B = IMG * IMG          # 784 block elements
FIX_CAP = P * NCORES   # 1024 sparse-fixup rows

F32 = mybir.dt.float32
BF16 = mybir.dt.bfloat16
OP = mybir.AluOpType
AF = mybir.ActivationFunctionType
NPBF = ml_dtypes.bfloat16

EPS_LOG = np.float32(1e-8)
RV_ALPHA = np.float32(0.9)


def _build_l1p(with_isr):
    """Heavy pass: um9 partials + passthrough output, one (som|rv) stream.

    with_isr=False (uniform rv): um = sum(d1^2)  (argmin is scale-invariant)
    with_isr=True  (general rv): um = sum((d1*isr)^2), isr = 1/sqrt(rv)
    shipped as a third block in the input row."""
    nb = 3 if with_isr else 2
    nc = bacc.Bacc("TRN2", num_devices=NCORES, debug=False)
    sv_d = nc.dram_tensor("sv", [UPC, nb * B], BF16, kind="ExternalInput")
    xb_d = nc.dram_tensor("xb", [P, B], BF16, kind="ExternalInput")
    out_d = nc.dram_tensor("out_t", [UPC, 2 * B], BF16, kind="ExternalOutput")
    um_d = nc.dram_tensor("um", [P, NT], F32, kind="ExternalOutput")

    with tile.TileContext(nc) as tc:
        with (
            tc.tile_pool(name="io", bufs=6) as io,
            tc.tile_pool(name="wk", bufs=4) as wk,
            tc.tile_pool(name="outp", bufs=4) as outp,
            tc.tile_pool(name="sm", bufs=1) as sm,
        ):
            xb = sm.tile([P, B], BF16, tag="xb")
            nc.scalar.dma_start(out=xb[:], in_=xb_d[:, :])
            um9 = sm.tile([P, NT], F32, tag="um9")
            for T in range(NT):
                r0 = P * T
                ti = io.tile([P, nb * B], BF16, tag="sv")
                nc.sync.dma_start(out=ti[:], in_=sv_d[r0:r0 + P, :])
                s_t, v_t = ti[:, 0:B], ti[:, B:2 * B]
                ot = outp.tile([P, 2 * B], BF16, tag="ot")
                nc.vector.tensor_scalar(out=ot[:, 0:B], in0=s_t,
                                        scalar1=0.0, scalar2=1.0,
                                        op0=OP.max, op1=OP.min)
                nc.vector.tensor_copy(ot[:, B:2 * B], v_t)
                d1 = wk.tile([P, B], BF16, tag="d1")
                nc.vector.tensor_tensor(d1[:], s_t, xb[:], OP.subtract)
                if with_isr:
                    t2 = wk.tile([P, B], BF16, tag="t2")
                    nc.vector.tensor_tensor(t2[:], d1[:], ti[:, 2 * B:3 * B],
                                            OP.mult)
                    d1 = t2
                scr = wk.tile([P, B], BF16, tag="scr")
                nc.scalar.activation(scr[:], d1[:], AF.Square,
                                     accum_out=um9[:, T:T + 1])
                nc.gpsimd.dma_start(out=out_d[r0:r0 + P, :], in_=ot[:])
            nc.sync.dma_start(out=um_d[:, :], in_=um9[:])
    nc.finalize()
    return nc


def _build_l1pu2():
    """Uniform-rv heavy pass: som-only input stream.  The var-plane
    passthrough is a constant fill (var_new = rv exactly for out-of-radius
    units, and rv is uniform), so instead of reading 1.6MB of rv just to
    write the same bytes back, the runtime scalar arrives as a [P,1] input
    and is broadcast on-device into one SBUF tile that feeds all nine
    var-plane writes.  Halves L1p input traffic: DMA busy 20.7 -> 15.7us.
    var-plane out-DMAs ride the scalar/HWDGE queue (gpsimd would serialize
    them against the som-plane out-DMAs on the SWDGE engine)."""
    nc = bacc.Bacc("TRN2", num_devices=NCORES, debug=False)
    s_d = nc.dram_tensor("s", [UPC, B], BF16, kind="ExternalInput")
    xb_d = nc.dram_tensor("xb", [P, B], BF16, kind="ExternalInput")
    rvv_d = nc.dram_tensor("rvv", [P, 1], F32, kind="ExternalInput")
    out_d = nc.dram_tensor("out_t", [UPC, 2 * B], BF16, kind="ExternalOutput")
    um_d = nc.dram_tensor("um", [P, NT], F32, kind="ExternalOutput")

    with tile.TileContext(nc) as tc:
        with (
            tc.tile_pool(name="io", bufs=6) as io,
            tc.tile_pool(name="wk", bufs=4) as wk,
            tc.tile_pool(name="outp", bufs=4) as outp,
            tc.tile_pool(name="sm", bufs=1) as sm,
        ):
            xb = sm.tile([P, B], BF16, tag="xb")
            nc.scalar.dma_start(out=xb[:], in_=xb_d[:, :])
            rvv = sm.tile([P, 1], F32, tag="rvv")
            nc.scalar.dma_start(out=rvv[:], in_=rvv_d[:, :])
            vhalf = sm.tile([P, B], BF16, tag="vhalf")
            nc.vector.tensor_scalar(out=vhalf[:], in0=xb[:], scalar1=0.0,
                                    scalar2=rvv[:], op0=OP.mult, op1=OP.add)
            um9 = sm.tile([P, NT], F32, tag="um9")
            for T in range(NT):
                r0 = P * T
                ti = io.tile([P, B], BF16, tag="s")
                nc.sync.dma_start(out=ti[:], in_=s_d[r0:r0 + P, :])
                ot = outp.tile([P, B], BF16, tag="ot")
                nc.vector.tensor_scalar(out=ot[:], in0=ti[:], scalar1=0.0,
                                        scalar2=1.0, op0=OP.max, op1=OP.min)
                d1 = wk.tile([P, B], BF16, tag="d1")
                nc.vector.tensor_tensor(d1[:], ti[:], xb[:], OP.subtract)
                scr = wk.tile([P, B], BF16, tag="scr")
                nc.scalar.activation(scr[:], d1[:], AF.Square,
                                     accum_out=um9[:, T:T + 1])
                nc.gpsimd.dma_start(out=out_d[r0:r0 + P, 0:B], in_=ot[:])
                nc.scalar.dma_start(out=out_d[r0:r0 + P, B:2 * B],
                                    in_=vhalf[:])
            nc.sync.dma_start(out=um_d[:, :], in_=um9[:])
    nc.finalize()
    return nc


def _build_l2s():
    """Sparse fixup: one [128, 1568] tile = 128 affected unit rows with
    per-partition (per-unit) factors fn = -fm, va, sg = sqrt((1-va)(1-fm)^2):
      som_new = (d1 * fn) + som;  var_new = Square(d1 * sg) + va * rv."""
    nc = bacc.Bacc("TRN2", num_devices=NCORES, debug=False)
    svx_d = nc.dram_tensor("svx", [P, 2 * B], BF16, kind="ExternalInput")
    xb_d = nc.dram_tensor("xb", [P, B], BF16, kind="ExternalInput")
    fvg_d = nc.dram_tensor("fvg", [P, 3], F32, kind="ExternalInput")
    outx_d = nc.dram_tensor("outx", [P, 2 * B], BF16, kind="ExternalOutput")

    with tile.TileContext(nc) as tc:
        with (
            tc.tile_pool(name="sm", bufs=1) as sm,
        ):
            xb = sm.tile([P, B], BF16, tag="xb")
            nc.scalar.dma_start(out=xb[:], in_=xb_d[:, :])
            fvg = sm.tile([P, 3], F32, tag="fvg")
            nc.scalar.dma_start(out=fvg[:], in_=fvg_d[:, :])
            ti = sm.tile([P, 2 * B], BF16, tag="svx")
            nc.sync.dma_start(out=ti[:], in_=svx_d[:, :])
            s_t, v_t = ti[:, 0:B], ti[:, B:2 * B]
            av = sm.tile([P, B], BF16, tag="av")
            nc.vector.tensor_scalar(out=av[:], in0=v_t, scalar1=fvg[:, 1:2],
                                    scalar2=None, op0=OP.mult)
            d1 = sm.tile([P, B], BF16, tag="d1")
            nc.vector.tensor_tensor(d1[:], s_t, xb[:], OP.subtract)
            q2 = sm.tile([P, B], BF16, tag="q2")
            nc.scalar.activation(q2[:], d1[:], AF.Square, scale=fvg[:, 2:3])
            m1 = sm.tile([P, B], BF16, tag="m1")
            nc.vector.tensor_scalar(out=m1[:], in0=d1[:], scalar1=fvg[:, 0:1],
                                    scalar2=None, op0=OP.mult)
            ot = sm.tile([P, 2 * B], BF16, tag="ot")
            nc.vector.tensor_tensor(ot[:, 0:B], m1[:], s_t, OP.add)
            nc.gpsimd.dma_start(out=outx_d[:, 0:B], in_=ot[:, 0:B])
            nc.vector.tensor_tensor(ot[:, B:2 * B], q2[:], av[:], OP.add)
            nc.gpsimd.dma_start(out=outx_d[:, B:2 * B], in_=ot[:, B:2 * B])
    nc.finalize()
    return nc


def _build_l2su():
    """Uniform-rv sparse fixup: rv is a runtime scalar, so va*rv is folded
    on host into a per-partition constant avp — no var-half input and no
    av op.  Input is the som rows only:
      som_new = (d1 * fn) + som;  var_new = Square(d1 * sg) + avp."""
    nc = bacc.Bacc("TRN2", num_devices=NCORES, debug=False)
    sx_d = nc.dram_tensor("sx", [P, B], BF16, kind="ExternalInput")
    xb_d = nc.dram_tensor("xb", [P, B], BF16, kind="ExternalInput")
    fvg_d = nc.dram_tensor("fvg", [P, 3], F32, kind="ExternalInput")
    outx_d = nc.dram_tensor("outx", [P, 2 * B], BF16, kind="ExternalOutput")

    with tile.TileContext(nc) as tc:
        with (
            tc.tile_pool(name="sm", bufs=1) as sm,
        ):
            xb = sm.tile([P, B], BF16, tag="xb")
            nc.scalar.dma_start(out=xb[:], in_=xb_d[:, :])
            fvg = sm.tile([P, 3], F32, tag="fvg")
            nc.scalar.dma_start(out=fvg[:], in_=fvg_d[:, :])
            ti = sm.tile([P, B], BF16, tag="sx")
            nc.sync.dma_start(out=ti[:], in_=sx_d[:, :])
            d1 = sm.tile([P, B], BF16, tag="d1")
            nc.vector.tensor_tensor(d1[:], ti[:], xb[:], OP.subtract)
            q2 = sm.tile([P, B], BF16, tag="q2")
            nc.scalar.activation(q2[:], d1[:], AF.Square, scale=fvg[:, 2:3])
            m1 = sm.tile([P, B], BF16, tag="m1")
            nc.vector.tensor_scalar(out=m1[:], in0=d1[:], scalar1=fvg[:, 0:1],
                                    scalar2=None, op0=OP.mult)
            ot = sm.tile([P, 2 * B], BF16, tag="ot")
            nc.vector.tensor_tensor(ot[:, 0:B], m1[:], ti[:], OP.add)
            nc.gpsimd.dma_start(out=outx_d[:, 0:B], in_=ot[:, 0:B])
            nc.vector.tensor_scalar(out=ot[:, B:2 * B], in0=q2[:],
                                    scalar1=fvg[:, 1:2], scalar2=None,
                                    op0=OP.add)
            nc.scalar.dma_start(out=outx_d[:, B:2 * B], in_=ot[:, B:2 * B])
    nc.finalize()
    return nc


def _build_l2d():
    """Dense fallback update over all units (per-tile-column factors)."""
    nc = bacc.Bacc("TRN2", num_devices=NCORES, debug=False)
    sv_d = nc.dram_tensor("sv", [UPC, 2 * B], BF16, kind="ExternalInput")
    xb_d = nc.dram_tensor("xb", [P, B], BF16, kind="ExternalInput")
    fvg_d = nc.dram_tensor("fvg", [P, 3 * NT], F32, kind="ExternalInput")
    out_d = nc.dram_tensor("out_t", [UPC, 2 * B], BF16, kind="ExternalOutput")

    with tile.TileContext(nc) as tc:
        with (
            tc.tile_pool(name="io", bufs=6) as io,
            tc.tile_pool(name="wk", bufs=4) as wk,
            tc.tile_pool(name="outp", bufs=4) as outp,
            tc.tile_pool(name="sm", bufs=1) as sm,
        ):
            xb = sm.tile([P, B], BF16, tag="xb")
            nc.scalar.dma_start(out=xb[:], in_=xb_d[:, :])
            fvg = sm.tile([P, 3 * NT], F32, tag="fvg")
            nc.scalar.dma_start(out=fvg[:], in_=fvg_d[:, :])
            fn = fvg[:, 0:NT]
            va = fvg[:, NT:2 * NT]
            sg = fvg[:, 2 * NT:3 * NT]
            for T in range(NT):
                r0 = P * T
                ti = io.tile([P, 2 * B], BF16, tag="sv")
                nc.sync.dma_start(out=ti[:], in_=sv_d[r0:r0 + P, :])
                s_t, v_t = ti[:, 0:B], ti[:, B:2 * B]
                d1 = wk.tile([P, B], BF16, tag="d1")
                nc.vector.tensor_tensor(d1[:], s_t, xb[:], OP.subtract)
                m1 = wk.tile([P, B], BF16, tag="m1")
                nc.scalar.mul(m1[:], d1[:], fn[:, T:T + 1])
                q2 = wk.tile([P, B], BF16, tag="q2")
                nc.scalar.activation(q2[:], d1[:], AF.Square,
                                     scale=sg[:, T:T + 1])
                av = wk.tile([P, B], BF16, tag="av")
                nc.vector.tensor_scalar(out=av[:], in0=v_t,
                                        scalar1=va[:, T:T + 1], scalar2=None,
                                        op0=OP.mult)
                ot = outp.tile([P, 2 * B], BF16, tag="ot")
                nc.vector.tensor_tensor(ot[:, 0:B], m1[:], s_t, OP.add)
                nc.vector.tensor_tensor(ot[:, B:2 * B], q2[:], av[:], OP.add)
                nc.gpsimd.dma_start(out=out_d[r0:r0 + P, :], in_=ot[:])
    nc.finalize()
    return nc


_CACHE = {}


def _get(name, builder, *args):
    if name not in _CACHE:
        _CACHE[name] = builder(*args)
    return _CACHE[name]


def _unit_major(a):
    """[S, S] -> [9216, 784]: one 28x28 block per row."""
    return (np.ascontiguousarray(a).reshape(N, IMG, N, IMG)
            .transpose(0, 2, 1, 3).reshape(UNITS, B))


def kernel(som, running_variance, learning_rates, radius,
           cartesian_distances, x):
    som = np.asarray(som, np.float32)
    rv = np.asarray(running_variance, np.float32)
    lr = np.asarray(learning_rates, np.float32)
    rad = np.asarray(radius, np.float32)
    cd = np.asarray(cartesian_distances, np.float32)
    x = np.asarray(x, np.float32)

    som_t = _unit_major(som)
    som_b = som_t.astype(NPBF)
    xb = np.broadcast_to(x.reshape(1, B), (P, B)).astype(NPBF)
    xb = np.ascontiguousarray(xb)

    uniform = bool((rv == rv.flat[0]).all()) and rv.flat[0] > 0
    if uniform:
        rvv = np.full((P, 1), rv.flat[0], np.float32)
        l1_maps = [{
            "s": np.ascontiguousarray(som_b[UPC * c:UPC * (c + 1)]),
            "xb": xb,
            "rvv": rvv,
        } for c in range(NCORES)]
        res1 = bass_utils.run_bass_kernel_spmd(
            _get("l1pu2", _build_l1pu2), l1_maps,
            core_ids=list(range(NCORES)))

        def rv_rows(rows):
            return np.full((len(rows), B), rv.flat[0], NPBF)
    else:
        rv_t = _unit_major(rv)
        rv_b = rv_t.astype(NPBF)
        isr_b = (np.float32(1.0) / np.sqrt(rv_t)).astype(NPBF)
        sv_full = np.concatenate([som_b, rv_b, isr_b], axis=1)
        l1_maps = [{
            "sv": np.ascontiguousarray(sv_full[UPC * c:UPC * (c + 1)]),
            "xb": xb,
        } for c in range(NCORES)]
        res1 = bass_utils.run_bass_kernel_spmd(
            _get("l1pg", _build_l1p, True), l1_maps,
            core_ids=list(range(NCORES)))

        def rv_rows(rows):
            return rv_b[rows]

    # passthrough output (exact for every out-of-radius unit) + um partials
    out_t = np.concatenate(
        [np.asarray(res1.results[c]["out_t"]) for c in range(NCORES)], axis=0)
    um = np.concatenate(
        [np.asarray(res1.results[c]["um"], np.float32).T.reshape(-1)
         for c in range(NCORES)])

    # ---- host glue: argmin + neighborhood factor maps (numpy f32) ----
    g = int(np.argmin(um))
    bi, bj = g // N, g % N
    r = rad[bi, bj]
    lr_s = lr[bi, bj]
    dist_mod = np.float32(1.0) / (np.float32(2.0) * r * r)
    constant = -np.log(EPS_LOG / lr_s) / dist_mod
    d = cd[:, :, bi, bj]
    mask = np.where(d > r, np.float32(0.0), np.float32(1.0))
    fm = mask * lr * np.exp(-d * dist_mod)
    va = RV_ALPHA - np.float32(0.5) + np.float32(1.0) / (
        np.float32(1.0) + np.exp(-d / constant))
    va = np.clip(va * mask + (np.float32(1.0) - mask),
                 np.float32(0.0), np.float32(1.0))
    fn_m = (-fm).reshape(-1).astype(np.float32)
    va_m = va.reshape(-1).astype(np.float32)
    sg_m = np.sqrt((np.float32(1.0) - va) * (np.float32(1.0) - fm) ** 2
                   ).reshape(-1).astype(np.float32)

    idx = np.flatnonzero(mask.reshape(-1) > 0)
    if idx.size == 0:
        pass  # empty neighborhood: passthrough IS the full update
    elif idx.size <= FIX_CAP:
        # ---- sparse fixup: <=1024 affected rows, 128 per core ----
        k = idx.size
        idx_pad = np.concatenate(
            [idx, np.full(FIX_CAP - k, idx[0], np.int64)])
        fvg = np.zeros((FIX_CAP, 3), np.float32)
        fvg[:k, 0] = fn_m[idx]
        fvg[:k, 2] = sg_m[idx]
        if uniform:
            # va*rv folds into a per-partition constant; som rows only
            fvg[:k, 1] = va_m[idx] * np.float32(rv.flat[0])
            fvg[k:, 1] = np.float32(rv.flat[0])
            l2_maps = [{
                "sx": np.ascontiguousarray(som_b[idx_pad[P * c:P * (c + 1)]]),
                "xb": xb,
                "fvg": np.ascontiguousarray(fvg[P * c:P * (c + 1)]),
            } for c in range(NCORES)]
            nc2 = _get("l2su", _build_l2su)
        else:
            fvg[:k, 1] = va_m[idx]
            fvg[k:, 1] = 1.0    # padding rows: identity update
            svx = np.concatenate([som_b[idx_pad], rv_rows(idx_pad)], axis=1)
            l2_maps = [{
                "svx": np.ascontiguousarray(svx[P * c:P * (c + 1)]),
                "xb": xb,
                "fvg": np.ascontiguousarray(fvg[P * c:P * (c + 1)]),
            } for c in range(NCORES)]
            nc2 = _get("l2s", _build_l2s)
        res2 = bass_utils.run_bass_kernel_spmd(
            nc2, l2_maps, core_ids=list(range(NCORES)))
        fix = np.concatenate(
            [np.asarray(res2.results[c]["outx"]) for c in range(NCORES)],
            axis=0)
        out_t[idx] = fix[:k]
    else:
        # ---- dense fallback: update every unit ----
        def shard(vec, c):
            return vec[UPC * c:UPC * (c + 1)].reshape(NT, P).T.copy()
        all_rows = np.arange(UNITS)
        sv2 = np.concatenate([som_b, rv_rows(all_rows)], axis=1)
        l2_maps = [{
            "sv": np.ascontiguousarray(sv2[UPC * c:UPC * (c + 1)]),
            "xb": xb,
            "fvg": np.ascontiguousarray(np.concatenate(
                [shard(fn_m, c), shard(va_m, c), shard(sg_m, c)], axis=1)),
        } for c in range(NCORES)]
        res2 = bass_utils.run_bass_kernel_spmd(
            _get("l2d", _build_l2d), l2_maps, core_ids=list(range(NCORES)))
        out_t = np.concatenate(
            [np.asarray(res2.results[c]["out_t"]) for c in range(NCORES)],
            axis=0)

    out_t = out_t.astype(np.float32)
    sn_t, vn_t = out_t[:, 0:B], out_t[:, B:2 * B]

    def untile(a):
        return (a.reshape(N, N, IMG, IMG).transpose(0, 2, 1, 3)
                .reshape(S, S))

    return np.stack([untile(sn_t), untile(vn_t)]).astype(np.float32)


# revision 7
# speedup vs baseline: 1.1709x; 1.0116x over previous
"""SOM update step on 8 Trainium2 NeuronCores — passthrough + sparse fixup.

Two collective-free launches (the baseline's mid-kernel AllGather cost
~1.05ms of barrier/transport time; see kernel_twopass_dense.py for the
previous dense two-pass version, sim ~42us):

  L1p (heavy, memory-roofline): stream (som|rv) bf16 once per core.
    Per tile, overlapped under the DMA stream: um9 BMU partial reduction
    (d1 = som - x, Square-with-accumulate), plus the full passthrough
    output (clip(som,0,1) | rv) -> out_t.  For every unit outside the BMU
    radius the reference update is exactly the identity (fm=0 -> som_new =
    clip(som); va=1 -> var_new = rv), so this IS the correct output there.
    Total traffic 6.4MB/core = the single-pass memory roofline.
  Host glue (O(units)): argmin over the 8x1152 device-computed distances,
    neighborhood factor maps in numpy f32 (using the real
    cartesian_distances input), affected-unit list (mask > 0, <=441 for
    r=10), gather those (som|rv) rows densely per core.
  L2s (tiny): one [128, 1568] tile per core, per-partition(=per-unit)
    factors, fused update, write back; host overlays the <=441 corrected
    rows over L1p's passthrough.  If the radius ever covers more than the
    1024-row capacity, fall back to the dense update launch (L2d) over
    all units.

bf16 I/O halves HBM traffic; validated against the 2e-2 rel-err gate at
1.13e-2 (worst case, in-disk units), deterministic.  The BMU argmin gap
(~2.1) is ~7x the worst-case bf16 perturbation of any unit distance.
"""
import numpy as np
import ml_dtypes

import concourse.bacc as bacc
import concourse.tile as tile
import concourse.bass_utils as bass_utils
from concourse import mybir

IMG = 28
N = 96
S = IMG * N            # 2688
NCORES = 8
UNITS = N * N          # 9216
UPC = UNITS // NCORES  # 1152 units per core
P = 128                # SBUF partitions
NT = UPC // P          # 9 tiles per core
B = IMG * IMG          # 784 block elements
FIX_CAP = P * NCORES   # 1024 sparse-fixup rows

F32 = mybir.dt.float32
BF16 = mybir.dt.bfloat16
OP = mybir.AluOpType
AF = mybir.ActivationFunctionType
NPBF = ml_dtypes.bfloat16

EPS_LOG = np.float32(1e-8)
RV_ALPHA = np.float32(0.9)


def _build_l1p(with_isr):
    """Heavy pass: um9 partials + passthrough output, one (som|rv) stream.

    with_isr=False (uniform rv): um = sum(d1^2)  (argmin is scale-invariant)
    with_isr=True  (general rv): um = sum((d1*isr)^2), isr = 1/sqrt(rv)
    shipped as a third block in the input row."""
    nb = 3 if with_isr else 2
    nc = bacc.Bacc("TRN2", num_devices=NCORES, debug=False)
    sv_d = nc.dram_tensor("sv", [UPC, nb * B], BF16, kind="ExternalInput")
    xb_d = nc.dram_tensor("xb", [P, B], BF16, kind="ExternalInput")
    out_d = nc.dram_tensor("out_t", [UPC, 2 * B], BF16, kind="ExternalOutput")
    um_d = nc.dram_tensor("um", [P, NT], F32, kind="ExternalOutput")

    with tile.TileContext(nc) as tc:
        with (
            tc.tile_pool(name="io", bufs=6) as io,
            tc.tile_pool(name="wk", bufs=4) as wk,
            tc.tile_pool(name="outp", bufs=4) as outp,
            tc.tile_pool(name="sm", bufs=1) as sm,
        ):
            xb = sm.tile([P, B], BF16, tag="xb")
            nc.scalar.dma_start(out=xb[:], in_=xb_d[:, :])
            um9 = sm.tile([P, NT], F32, tag="um9")
            for T in range(NT):
                r0 = P * T
                ti = io.tile([P, nb * B], BF16, tag="sv")
                nc.sync.dma_start(out=ti[:], in_=sv_d[r0:r0 + P, :])
                s_t, v_t = ti[:, 0:B], ti[:, B:2 * B]
                ot = outp.tile([P, 2 * B], BF16, tag="ot")
                nc.vector.tensor_scalar(out=ot[:, 0:B], in0=s_t,
                                        scalar1=0.0, scalar2=1.0,
                                        op0=OP.max, op1=OP.min)
                nc.vector.tensor_copy(ot[:, B:2 * B], v_t)
                d1 = wk.tile([P, B], BF16, tag="d1")
                nc.vector.tensor_tensor(d1[:], s_t, xb[:], OP.subtract)
                if with_isr:
                    t2 = wk.tile([P, B], BF16, tag="t2")
                    nc.vector.tensor_tensor(t2[:], d1[:], ti[:, 2 * B:3 * B],
                                            OP.mult)
                    d1 = t2
                scr = wk.tile([P, B], BF16, tag="scr")
                nc.scalar.activation(scr[:], d1[:], AF.Square,
                                     accum_out=um9[:, T:T + 1])
                nc.gpsimd.dma_start(out=out_d[r0:r0 + P, :], in_=ot[:])
            nc.sync.dma_start(out=um_d[:, :], in_=um9[:])
    nc.finalize()
    return nc


def _build_l1pu2():
    """Uniform-rv heavy pass: som-only input stream.  The var-plane
    passthrough is a constant fill (var_new = rv exactly for out-of-radius
    units, and rv is uniform), so instead of reading 1.6MB of rv just to
    write the same bytes back, the runtime scalar arrives as a [P,1] input
    and is broadcast on-device into one SBUF tile that feeds all nine
    var-plane writes.  Halves L1p input traffic: DMA busy 20.7 -> 15.7us.
    var-plane out-DMAs ride the scalar/HWDGE queue (gpsimd would serialize
    them against the som-plane out-DMAs on the SWDGE engine)."""
    nc = bacc.Bacc("TRN2", num_devices=NCORES, debug=False)
    s_d = nc.dram_tensor("s", [UPC, B], BF16, kind="ExternalInput")
    xb_d = nc.dram_tensor("xb", [P, B], BF16, kind="ExternalInput")
    rvv_d = nc.dram_tensor("rvv", [P, 1], F32, kind="ExternalInput")
    out_d = nc.dram_tensor("out_t", [UPC, 2 * B], BF16, kind="ExternalOutput")
    um_d = nc.dram_tensor("um", [P, NT], F32, kind="ExternalOutput")

    with tile.TileContext(nc) as tc:
        with (
            tc.tile_pool(name="io", bufs=6) as io,
            tc.tile_pool(name="wk", bufs=4) as wk,
            tc.tile_pool(name="outp", bufs=4) as outp,
            tc.tile_pool(name="sm", bufs=1) as sm,
        ):
            xb = sm.tile([P, B], BF16, tag="xb")
            nc.scalar.dma_start(out=xb[:], in_=xb_d[:, :])
            rvv = sm.tile([P, 1], F32, tag="rvv")
            nc.scalar.dma_start(out=rvv[:], in_=rvv_d[:, :])
            vhalf = sm.tile([P, B], BF16, tag="vhalf")
            nc.vector.tensor_scalar(out=vhalf[:], in0=xb[:], scalar1=0.0,
                                    scalar2=rvv[:], op0=OP.mult, op1=OP.add)
            um9 = sm.tile([P, NT], F32, tag="um9")
            for T in range(NT):
                r0 = P * T
                ti = io.tile([P, B], BF16, tag="s")
                nc.sync.dma_start(out=ti[:], in_=s_d[r0:r0 + P, :])
                ot = outp.tile([P, B], BF16, tag="ot")
                nc.vector.tensor_scalar(out=ot[:], in0=ti[:], scalar1=0.0,
                                        scalar2=1.0, op0=OP.max, op1=OP.min)
                d1 = wk.tile([P, B], BF16, tag="d1")
                nc.vector.tensor_tensor(d1[:], ti[:], xb[:], OP.subtract)
                scr = wk.tile([P, B], BF16, tag="scr")
                nc.scalar.activation(scr[:], d1[:], AF.Square,
                                     accum_out=um9[:, T:T + 1])
                nc.gpsimd.dma_start(out=out_d[r0:r0 + P, 0:B], in_=ot[:])
                nc.scalar.dma_start(out=out_d[r0:r0 + P, B:2 * B],
                                    in_=vhalf[:])
            nc.sync.dma_start(out=um_d[:, :], in_=um9[:])
    nc.finalize()
    return nc


def _build_l2s():
    """Sparse fixup: one [128, 1568] tile = 128 affected unit rows with
    per-partition (per-unit) factors fn = -fm, va, sg = sqrt((1-va)(1-fm)^2):
      som_new = (d1 * fn) + som;  var_new = Square(d1 * sg) + va * rv."""
    nc = bacc.Bacc("TRN2", num_devices=NCORES, debug=False)
    svx_d = nc.dram_tensor("svx", [P, 2 * B], BF16, kind="ExternalInput")
    xb_d = nc.dram_tensor("xb", [P, B], BF16, kind="ExternalInput")
    fvg_d = nc.dram_tensor("fvg", [P, 3], F32, kind="ExternalInput")
    outx_d = nc.dram_tensor("outx", [P, 2 * B], BF16, kind="ExternalOutput")

    with tile.TileContext(nc) as tc:
        with (
            tc.tile_pool(name="sm", bufs=1) as sm,
        ):
            xb = sm.tile([P, B], BF16, tag="xb")
            nc.scalar.dma_start(out=xb[:], in_=xb_d[:, :])
            fvg = sm.tile([P, 3], F32, tag="fvg")
            nc.scalar.dma_start(out=fvg[:], in_=fvg_d[:, :])
            ti = sm.tile([P, 2 * B], BF16, tag="svx")
            nc.sync.dma_start(out=ti[:], in_=svx_d[:, :])
            s_t, v_t = ti[:, 0:B], ti[:, B:2 * B]
            av = sm.tile([P, B], BF16, tag="av")
            nc.vector.tensor_scalar(out=av[:], in0=v_t, scalar1=fvg[:, 1:2],
                                    scalar2=None, op0=OP.mult)
            d1 = sm.tile([P, B], BF16, tag="d1")
            nc.vector.tensor_tensor(d1[:], s_t, xb[:], OP.subtract)
            q2 = sm.tile([P, B], BF16, tag="q2")
            nc.scalar.activation(q2[:], d1[:], AF.Square, scale=fvg[:, 2:3])
            m1 = sm.tile([P, B], BF16, tag="m1")
            nc.vector.tensor_scalar(out=m1[:], in0=d1[:], scalar1=fvg[:, 0:1],
                                    scalar2=None, op0=OP.mult)
            ot = sm.tile([P, 2 * B], BF16, tag="ot")
            nc.vector.tensor_tensor(ot[:, 0:B], m1[:], s_t, OP.add)
            nc.gpsimd.dma_start(out=outx_d[:, 0:B], in_=ot[:, 0:B])
            nc.vector.tensor_tensor(ot[:, B:2 * B], q2[:], av[:], OP.add)
            nc.gpsimd.dma_start(out=outx_d[:, B:2 * B], in_=ot[:, B:2 * B])
    nc.finalize()
    return nc


def _build_l2su():
    """Uniform-rv sparse fixup: rv is a runtime scalar, so va*rv is folded
    on host into a per-partition constant avp — no var-half input and no
    av op.  Input is the som rows only:
      som_new = (d1 * fn) + som;  var_new = Square(d1 * sg) + avp."""
    nc = bacc.Bacc("TRN2", num_devices=NCORES, debug=False)
    sx_d = nc.dram_tensor("sx", [P, B], BF16, kind="ExternalInput")
    xb_d = nc.dram_tensor("xb", [P, B], BF16, kind="ExternalInput")
    fvg_d = nc.dram_tensor("fvg", [P, 3], F32, kind="ExternalInput")
    outx_d = nc.dram_tensor("outx", [P, 2 * B], BF16, kind="ExternalOutput")

    with tile.TileContext(nc) as tc:
        with (
            tc.tile_pool(name="sm", bufs=1) as sm,
        ):
            xb = sm.tile([P, B], BF16, tag="xb")
            nc.scalar.dma_start(out=xb[:], in_=xb_d[:, :])
            fvg = sm.tile([P, 3], F32, tag="fvg")
            nc.scalar.dma_start(out=fvg[:], in_=fvg_d[:, :])
            ti = sm.tile([P, B], BF16, tag="sx")
            nc.sync.dma_start(out=ti[:], in_=sx_d[:, :])
            d1 = sm.tile([P, B], BF16, tag="d1")
            nc.vector.tensor_tensor(d1[:], ti[:], xb[:], OP.subtract)
            q2 = sm.tile([P, B], BF16, tag="q2")
            nc.scalar.activation(q2[:], d1[:], AF.Square, scale=fvg[:, 2:3])
            m1 = sm.tile([P, B], BF16, tag="m1")
            nc.vector.tensor_scalar(out=m1[:], in0=d1[:], scalar1=fvg[:, 0:1],
                                    scalar2=None, op0=OP.mult)
            ot = sm.tile([P, 2 * B], BF16, tag="ot")
            nc.vector.tensor_tensor(ot[:, 0:B], m1[:], ti[:], OP.add)
            nc.sync.dma_start(out=outx_d[:, 0:B], in_=ot[:, 0:B])
            nc.vector.tensor_scalar(out=ot[:, B:2 * B], in0=q2[:],
                                    scalar1=fvg[:, 1:2], scalar2=None,
                                    op0=OP.add)
            nc.sync.dma_start(out=outx_d[:, B:2 * B], in_=ot[:, B:2 * B])
    nc.finalize()
    return nc


def _build_l2d():
    """Dense fallback update over all units (per-tile-column factors)."""
    nc = bacc.Bacc("TRN2", num_devices=NCORES, debug=False)
    sv_d = nc.dram_tensor("sv", [UPC, 2 * B], BF16, kind="ExternalInput")
    xb_d = nc.dram_tensor("xb", [P, B], BF16, kind="ExternalInput")
    fvg_d = nc.dram_tensor("fvg", [P, 3 * NT], F32, kind="ExternalInput")
    out_d = nc.dram_tensor("out_t", [UPC, 2 * B], BF16, kind="ExternalOutput")

    with tile.TileContext(nc) as tc:
        with (
            tc.tile_pool(name="io", bufs=6) as io,
            tc.tile_pool(name="wk", bufs=4) as wk,
            tc.tile_pool(name="outp", bufs=4) as outp,
            tc.tile_pool(name="sm", bufs=1) as sm,
        ):
            xb = sm.tile([P, B], BF16, tag="xb")
            nc.scalar.dma_start(out=xb[:], in_=xb_d[:, :])
            fvg = sm.tile([P, 3 * NT], F32, tag="fvg")
            nc.scalar.dma_start(out=fvg[:], in_=fvg_d[:, :])
            fn = fvg[:, 0:NT]
            va = fvg[:, NT:2 * NT]
            sg = fvg[:, 2 * NT:3 * NT]
            for T in range(NT):
                r0 = P * T
                ti = io.tile([P, 2 * B], BF16, tag="sv")
                nc.sync.dma_start(out=ti[:], in_=sv_d[r0:r0 + P, :])
                s_t, v_t = ti[:, 0:B], ti[:, B:2 * B]
                d1 = wk.tile([P, B], BF16, tag="d1")
                nc.vector.tensor_tensor(d1[:], s_t, xb[:], OP.subtract)
                m1 = wk.tile([P, B], BF16, tag="m1")
                nc.scalar.mul(m1[:], d1[:], fn[:, T:T + 1])
                q2 = wk.tile([P, B], BF16, tag="q2")
                nc.scalar.activation(q2[:], d1[:], AF.Square,
                                     scale=sg[:, T:T + 1])
                av = wk.tile([P, B], BF16, tag="av")
                nc.vector.tensor_scalar(out=av[:], in0=v_t,
                                        scalar1=va[:, T:T + 1], scalar2=None,
                                        op0=OP.mult)
                ot = outp.tile([P, 2 * B], BF16, tag="ot")
                nc.vector.tensor_tensor(ot[:, 0:B], m1[:], s_t, OP.add)
                nc.vector.tensor_tensor(ot[:, B:2 * B], q2[:], av[:], OP.add)
                nc.gpsimd.dma_start(out=out_d[r0:r0 + P, :], in_=ot[:])
    nc.finalize()
    return nc


_CACHE = {}


def _get(name, builder, *args):
    if name not in _CACHE:
        _CACHE[name] = builder(*args)
    return _CACHE[name]


def _unit_major(a):
    """[S, S] -> [9216, 784]: one 28x28 block per row."""
    return (np.ascontiguousarray(a).reshape(N, IMG, N, IMG)
            .transpose(0, 2, 1, 3).reshape(UNITS, B))


def kernel(som, running_variance, learning_rates, radius,
           cartesian_distances, x):
    som = np.asarray(som, np.float32)
    rv = np.asarray(running_variance, np.float32)
    lr = np.asarray(learning_rates, np.float32)
    rad = np.asarray(radius, np.float32)
    cd = np.asarray(cartesian_distances, np.float32)
    x = np.asarray(x, np.float32)

    som_t = _unit_major(som)
    som_b = som_t.astype(NPBF)
    xb = np.broadcast_to(x.reshape(1, B), (P, B)).astype(NPBF)
    xb = np.ascontiguousarray(xb)

    uniform = bool((rv == rv.flat[0]).all()) and rv.flat[0] > 0
    if uniform:
        rvv = np.full((P, 1), rv.flat[0], np.float32)
        l1_maps = [{
            "s": np.ascontiguousarray(som_b[UPC * c:UPC * (c + 1)]),
            "xb": xb,
            "rvv": rvv,
        } for c in range(NCORES)]
        res1 = bass_utils.run_bass_kernel_spmd(
            _get("l1pu2", _build_l1pu2), l1_maps,
            core_ids=list(range(NCORES)))

        def rv_rows(rows):
            return np.full((len(rows), B), rv.flat[0], NPBF)
    else:
        rv_t = _unit_major(rv)
        rv_b = rv_t.astype(NPBF)
        isr_b = (np.float32(1.0) / np.sqrt(rv_t)).astype(NPBF)
        sv_full = np.concatenate([som_b, rv_b, isr_b], axis=1)
        l1_maps = [{
            "sv": np.ascontiguousarray(sv_full[UPC * c:UPC * (c + 1)]),
            "xb": xb,
        } for c in range(NCORES)]
        res1 = bass_utils.run_bass_kernel_spmd(
            _get("l1pg", _build_l1p, True), l1_maps,
            core_ids=list(range(NCORES)))

        def rv_rows(rows):
            return rv_b[rows]

    # passthrough output (exact for every out-of-radius unit) + um partials
    out_t = np.concatenate(
        [np.asarray(res1.results[c]["out_t"]) for c in range(NCORES)], axis=0)
    um = np.concatenate(
        [np.asarray(res1.results[c]["um"], np.float32).T.reshape(-1)
         for c in range(NCORES)])

    # ---- host glue: argmin + neighborhood factor maps (numpy f32) ----
    g = int(np.argmin(um))
    bi, bj = g // N, g % N
    r = rad[bi, bj]
    lr_s = lr[bi, bj]
    dist_mod = np.float32(1.0) / (np.float32(2.0) * r * r)
    constant = -np.log(EPS_LOG / lr_s) / dist_mod
    d = cd[:, :, bi, bj]
    mask = np.where(d > r, np.float32(0.0), np.float32(1.0))
    fm = mask * lr * np.exp(-d * dist_mod)
    va = RV_ALPHA - np.float32(0.5) + np.float32(1.0) / (
        np.float32(1.0) + np.exp(-d / constant))
    va = np.clip(va * mask + (np.float32(1.0) - mask),
                 np.float32(0.0), np.float32(1.0))
    fn_m = (-fm).reshape(-1).astype(np.float32)
    va_m = va.reshape(-1).astype(np.float32)
    sg_m = np.sqrt((np.float32(1.0) - va) * (np.float32(1.0) - fm) ** 2
                   ).reshape(-1).astype(np.float32)

    idx = np.flatnonzero(mask.reshape(-1) > 0)
    if idx.size == 0:
        pass  # empty neighborhood: passthrough IS the full update
    elif idx.size <= FIX_CAP:
        # ---- sparse fixup: <=1024 affected rows, 128 per core ----
        k = idx.size
        idx_pad = np.concatenate(
            [idx, np.full(FIX_CAP - k, idx[0], np.int64)])
        fvg = np.zeros((FIX_CAP, 3), np.float32)
        fvg[:k, 0] = fn_m[idx]
        fvg[:k, 2] = sg_m[idx]
        if uniform:
            # va*rv folds into a per-partition constant; som rows only
            fvg[:k, 1] = va_m[idx] * np.float32(rv.flat[0])
            fvg[k:, 1] = np.float32(rv.flat[0])
            l2_maps = [{
                "sx": np.ascontiguousarray(som_b[idx_pad[P * c:P * (c + 1)]]),
                "xb": xb,
                "fvg": np.ascontiguousarray(fvg[P * c:P * (c + 1)]),
            } for c in range(NCORES)]
            nc2 = _get("l2su", _build_l2su)
        else:
            fvg[:k, 1] = va_m[idx]
            fvg[k:, 1] = 1.0    # padding rows: identity update
            svx = np.concatenate([som_b[idx_pad], rv_rows(idx_pad)], axis=1)
            l2_maps = [{
                "svx": np.ascontiguousarray(svx[P * c:P * (c + 1)]),
                "xb": xb,
                "fvg": np.ascontiguousarray(fvg[P * c:P * (c + 1)]),
            } for c in range(NCORES)]
            nc2 = _get("l2s", _build_l2s)
        res2 = bass_utils.run_bass_kernel_spmd(
            nc2, l2_maps, core_ids=list(range(NCORES)))
        fix = np.concatenate(
            [np.asarray(res2.results[c]["outx"]) for c in range(NCORES)],
            axis=0)
        out_t[idx] = fix[:k]
    else:
        # ---- dense fallback: update every unit ----
        def shard(vec, c):
            return vec[UPC * c:UPC * (c + 1)].reshape(NT, P).T.copy()
        all_rows = np.arange(UNITS)
        sv2 = np.concatenate([som_b, rv_rows(all_rows)], axis=1)
        l2_maps = [{
            "sv": np.ascontiguousarray(sv2[UPC * c:UPC * (c + 1)]),
            "xb": xb,
            "fvg": np.ascontiguousarray(np.concatenate(
                [shard(fn_m, c), shard(va_m, c), shard(sg_m, c)], axis=1)),
        } for c in range(NCORES)]
        res2 = bass_utils.run_bass_kernel_spmd(
            _get("l2d", _build_l2d), l2_maps, core_ids=list(range(NCORES)))
        out_t = np.concatenate(
            [np.asarray(res2.results[c]["out_t"]) for c in range(NCORES)],
            axis=0)

    out_t = out_t.astype(np.float32)
    sn_t, vn_t = out_t[:, 0:B], out_t[:, B:2 * B]

    def untile(a):
        return (a.reshape(N, N, IMG, IMG).transpose(0, 2, 1, 3)
                .reshape(S, S))

    return np.stack([untile(sn_t), untile(vn_t)]).astype(np.float32)


# revision 8
# speedup vs baseline: 1.1983x; 1.0234x over previous
"""SOM update step on 8 Trainium2 NeuronCores — passthrough + sparse fixup.

Two collective-free launches (the baseline's mid-kernel AllGather cost
~1.05ms of barrier/transport time; see kernel_twopass_dense.py for the
previous dense two-pass version, sim ~42us):

  L1p (heavy, memory-roofline): stream (som|rv) bf16 once per core.
    Per tile, overlapped under the DMA stream: um9 BMU partial reduction
    (d1 = som - x, Square-with-accumulate), plus the full passthrough
    output (clip(som,0,1) | rv) -> out_t.  For every unit outside the BMU
    radius the reference update is exactly the identity (fm=0 -> som_new =
    clip(som); va=1 -> var_new = rv), so this IS the correct output there.
    Total traffic 6.4MB/core = the single-pass memory roofline.
  Host glue (O(units)): argmin over the 8x1152 device-computed distances,
    neighborhood factor maps in numpy f32 (using the real
    cartesian_distances input), affected-unit list (mask > 0, <=441 for
    r=10), gather those (som|rv) rows densely per core.
  L2s (tiny): one [128, 1568] tile per core, per-partition(=per-unit)
    factors, fused update, write back; host overlays the <=441 corrected
    rows over L1p's passthrough.  If the radius ever covers more than the
    1024-row capacity, fall back to the dense update launch (L2d) over
    all units.

bf16 I/O halves HBM traffic; validated against the 2e-2 rel-err gate at
1.13e-2 (worst case, in-disk units), deterministic.  The BMU argmin gap
(~2.1) is ~7x the worst-case bf16 perturbation of any unit distance.
"""
import numpy as np
import ml_dtypes

import concourse.bacc as bacc
import concourse.tile as tile
import concourse.bass_utils as bass_utils
from concourse import mybir

IMG = 28
N = 96
S = IMG * N            # 2688
NCORES = 8
UNITS = N * N          # 9216
UPC = UNITS // NCORES  # 1152 units per core
P = 128                # SBUF partitions
NT = UPC // P          # 9 tiles per core
B = IMG * IMG          # 784 block elements
FIX_CAP = P * NCORES   # 1024 sparse-fixup rows

F32 = mybir.dt.float32
BF16 = mybir.dt.bfloat16
OP = mybir.AluOpType
AF = mybir.ActivationFunctionType
NPBF = ml_dtypes.bfloat16

EPS_LOG = np.float32(1e-8)
RV_ALPHA = np.float32(0.9)


def _build_l1p(with_isr):
    """Heavy pass: um9 partials + passthrough output, one (som|rv) stream.

    with_isr=False (uniform rv): um = sum(d1^2)  (argmin is scale-invariant)
    with_isr=True  (general rv): um = sum((d1*isr)^2), isr = 1/sqrt(rv)
    shipped as a third block in the input row."""
    nb = 3 if with_isr else 2
    nc = bacc.Bacc("TRN2", num_devices=NCORES, debug=False)
    sv_d = nc.dram_tensor("sv", [UPC, nb * B], BF16, kind="ExternalInput")
    xb_d = nc.dram_tensor("xb", [P, B], BF16, kind="ExternalInput")
    out_d = nc.dram_tensor("out_t", [UPC, 2 * B], BF16, kind="ExternalOutput")
    um_d = nc.dram_tensor("um", [P, NT], F32, kind="ExternalOutput")

    with tile.TileContext(nc) as tc:
        with (
            tc.tile_pool(name="io", bufs=6) as io,
            tc.tile_pool(name="wk", bufs=4) as wk,
            tc.tile_pool(name="outp", bufs=4) as outp,
            tc.tile_pool(name="sm", bufs=1) as sm,
        ):
            xb = sm.tile([P, B], BF16, tag="xb")
            nc.scalar.dma_start(out=xb[:], in_=xb_d[:, :])
            um9 = sm.tile([P, NT], F32, tag="um9")
            for T in range(NT):
                r0 = P * T
                ti = io.tile([P, nb * B], BF16, tag="sv")
                nc.sync.dma_start(out=ti[:], in_=sv_d[r0:r0 + P, :])
                s_t, v_t = ti[:, 0:B], ti[:, B:2 * B]
                ot = outp.tile([P, 2 * B], BF16, tag="ot")
                nc.vector.tensor_scalar(out=ot[:, 0:B], in0=s_t,
                                        scalar1=0.0, scalar2=1.0,
                                        op0=OP.max, op1=OP.min)
                nc.vector.tensor_copy(ot[:, B:2 * B], v_t)
                d1 = wk.tile([P, B], BF16, tag="d1")
                nc.vector.tensor_tensor(d1[:], s_t, xb[:], OP.subtract)
                if with_isr:
                    t2 = wk.tile([P, B], BF16, tag="t2")
                    nc.vector.tensor_tensor(t2[:], d1[:], ti[:, 2 * B:3 * B],
                                            OP.mult)
                    d1 = t2
                scr = wk.tile([P, B], BF16, tag="scr")
                nc.scalar.activation(scr[:], d1[:], AF.Square,
                                     accum_out=um9[:, T:T + 1])
                nc.gpsimd.dma_start(out=out_d[r0:r0 + P, :], in_=ot[:])
            nc.sync.dma_start(out=um_d[:, :], in_=um9[:])
    nc.finalize()
    return nc


def _build_l1pu2():
    """Uniform-rv heavy pass: som-only input stream.  The var-plane
    passthrough is a constant fill (var_new = rv exactly for out-of-radius
    units, and rv is uniform), so instead of reading 1.6MB of rv just to
    write the same bytes back, the runtime scalar arrives as a [P,1] input
    and is broadcast on-device into one SBUF tile that feeds all nine
    var-plane writes.  Halves L1p input traffic: DMA busy 20.7 -> 15.7us.
    var-plane out-DMAs ride the scalar/HWDGE queue (gpsimd would serialize
    them against the som-plane out-DMAs on the SWDGE engine)."""
    nc = bacc.Bacc("TRN2", num_devices=NCORES, debug=False)
    s_d = nc.dram_tensor("s", [UPC, B], BF16, kind="ExternalInput")
    xb_d = nc.dram_tensor("xb", [P, B], BF16, kind="ExternalInput")
    rvv_d = nc.dram_tensor("rvv", [P, 1], F32, kind="ExternalInput")
    out_d = nc.dram_tensor("out_t", [UPC, 2 * B], BF16, kind="ExternalOutput")
    um_d = nc.dram_tensor("um", [P, NT], F32, kind="ExternalOutput")

    with tile.TileContext(nc) as tc:
        with (
            tc.tile_pool(name="io", bufs=6) as io,
            tc.tile_pool(name="wk", bufs=4) as wk,
            tc.tile_pool(name="outp", bufs=4) as outp,
            tc.tile_pool(name="sm", bufs=1) as sm,
        ):
            xb = sm.tile([P, B], BF16, tag="xb")
            nc.scalar.dma_start(out=xb[:], in_=xb_d[:, :])
            rvv = sm.tile([P, 1], F32, tag="rvv")
            nc.scalar.dma_start(out=rvv[:], in_=rvv_d[:, :])
            vhalf = sm.tile([P, B], BF16, tag="vhalf")
            nc.vector.tensor_scalar(out=vhalf[:], in0=xb[:], scalar1=0.0,
                                    scalar2=rvv[:], op0=OP.mult, op1=OP.add)
            um9 = sm.tile([P, NT], F32, tag="um9")
            for T in range(NT):
                r0 = P * T
                ti = io.tile([P, B], BF16, tag="s")
                nc.sync.dma_start(out=ti[:], in_=s_d[r0:r0 + P, :])
                ot = outp.tile([P, B], BF16, tag="ot")
                nc.vector.tensor_scalar(out=ot[:], in0=ti[:], scalar1=0.0,
                                        scalar2=1.0, op0=OP.max, op1=OP.min)
                d1 = wk.tile([P, B], BF16, tag="d1")
                nc.vector.tensor_tensor(d1[:], ti[:], xb[:], OP.subtract)
                scr = wk.tile([P, B], BF16, tag="scr")
                nc.scalar.activation(scr[:], d1[:], AF.Square,
                                     accum_out=um9[:, T:T + 1])
                nc.scalar.dma_start(out=out_d[r0:r0 + P, 0:B], in_=ot[:])
                nc.gpsimd.dma_start(out=out_d[r0:r0 + P, B:2 * B],
                                    in_=vhalf[:])
            nc.sync.dma_start(out=um_d[:, :], in_=um9[:])
    nc.finalize()
    return nc


def _build_l2s():
    """Sparse fixup: one [128, 1568] tile = 128 affected unit rows with
    per-partition (per-unit) factors fn = -fm, va, sg = sqrt((1-va)(1-fm)^2):
      som_new = (d1 * fn) + som;  var_new = Square(d1 * sg) + va * rv."""
    nc = bacc.Bacc("TRN2", num_devices=NCORES, debug=False)
    svx_d = nc.dram_tensor("svx", [P, 2 * B], BF16, kind="ExternalInput")
    xb_d = nc.dram_tensor("xb", [P, B], BF16, kind="ExternalInput")
    fvg_d = nc.dram_tensor("fvg", [P, 3], F32, kind="ExternalInput")
    outx_d = nc.dram_tensor("outx", [P, 2 * B], BF16, kind="ExternalOutput")

    with tile.TileContext(nc) as tc:
        with (
            tc.tile_pool(name="sm", bufs=1) as sm,
        ):
            xb = sm.tile([P, B], BF16, tag="xb")
            nc.scalar.dma_start(out=xb[:], in_=xb_d[:, :])
            fvg = sm.tile([P, 3], F32, tag="fvg")
            nc.scalar.dma_start(out=fvg[:], in_=fvg_d[:, :])
            ti = sm.tile([P, 2 * B], BF16, tag="svx")
            nc.sync.dma_start(out=ti[:], in_=svx_d[:, :])
            s_t, v_t = ti[:, 0:B], ti[:, B:2 * B]
            av = sm.tile([P, B], BF16, tag="av")
            nc.vector.tensor_scalar(out=av[:], in0=v_t, scalar1=fvg[:, 1:2],
                                    scalar2=None, op0=OP.mult)
            d1 = sm.tile([P, B], BF16, tag="d1")
            nc.vector.tensor_tensor(d1[:], s_t, xb[:], OP.subtract)
            q2 = sm.tile([P, B], BF16, tag="q2")
            nc.scalar.activation(q2[:], d1[:], AF.Square, scale=fvg[:, 2:3])
            m1 = sm.tile([P, B], BF16, tag="m1")
            nc.vector.tensor_scalar(out=m1[:], in0=d1[:], scalar1=fvg[:, 0:1],
                                    scalar2=None, op0=OP.mult)
            ot = sm.tile([P, 2 * B], BF16, tag="ot")
            nc.vector.tensor_tensor(ot[:, 0:B], m1[:], s_t, OP.add)
            nc.gpsimd.dma_start(out=outx_d[:, 0:B], in_=ot[:, 0:B])
            nc.vector.tensor_tensor(ot[:, B:2 * B], q2[:], av[:], OP.add)
            nc.gpsimd.dma_start(out=outx_d[:, B:2 * B], in_=ot[:, B:2 * B])
    nc.finalize()
    return nc


def _build_l2su():
    """Uniform-rv sparse fixup: rv is a runtime scalar, so va*rv is folded
    on host into a per-partition constant avp — no var-half input and no
    av op.  Input is the som rows only:
      som_new = (d1 * fn) + som;  var_new = Square(d1 * sg) + avp."""
    nc = bacc.Bacc("TRN2", num_devices=NCORES, debug=False)
    sx_d = nc.dram_tensor("sx", [P, 2 * B], BF16, kind="ExternalInput")
    fvg_d = nc.dram_tensor("fvg", [P, 3], F32, kind="ExternalInput")
    outx_d = nc.dram_tensor("outx", [P, 2 * B], BF16, kind="ExternalOutput")

    with tile.TileContext(nc) as tc:
        with (
            tc.tile_pool(name="sm", bufs=1) as sm,
        ):
            fvg = sm.tile([P, 3], F32, tag="fvg")
            nc.scalar.dma_start(out=fvg[:], in_=fvg_d[:, :])
            ti = sm.tile([P, 2 * B], BF16, tag="sx")
            nc.sync.dma_start(out=ti[:], in_=sx_d[:, :])
            s_t, xb_t = ti[:, 0:B], ti[:, B:2 * B]
            d1 = sm.tile([P, B], BF16, tag="d1")
            nc.vector.tensor_tensor(d1[:], s_t, xb_t, OP.subtract)
            q2 = sm.tile([P, B], BF16, tag="q2")
            nc.scalar.activation(q2[:], d1[:], AF.Square, scale=fvg[:, 2:3])
            m1 = sm.tile([P, B], BF16, tag="m1")
            nc.vector.tensor_scalar(out=m1[:], in0=d1[:], scalar1=fvg[:, 0:1],
                                    scalar2=None, op0=OP.mult)
            ot = sm.tile([P, 2 * B], BF16, tag="ot")
            nc.vector.tensor_tensor(ot[:, 0:B], m1[:], s_t, OP.add)
            nc.sync.dma_start(out=outx_d[:, 0:B], in_=ot[:, 0:B])
            nc.vector.tensor_scalar(out=ot[:, B:2 * B], in0=q2[:],
                                    scalar1=fvg[:, 1:2], scalar2=None,
                                    op0=OP.add)
            nc.sync.dma_start(out=outx_d[:, B:2 * B], in_=ot[:, B:2 * B])
    nc.finalize()
    return nc


def _build_l2d():
    """Dense fallback update over all units (per-tile-column factors)."""
    nc = bacc.Bacc("TRN2", num_devices=NCORES, debug=False)
    sv_d = nc.dram_tensor("sv", [UPC, 2 * B], BF16, kind="ExternalInput")
    xb_d = nc.dram_tensor("xb", [P, B], BF16, kind="ExternalInput")
    fvg_d = nc.dram_tensor("fvg", [P, 3 * NT], F32, kind="ExternalInput")
    out_d = nc.dram_tensor("out_t", [UPC, 2 * B], BF16, kind="ExternalOutput")

    with tile.TileContext(nc) as tc:
        with (
            tc.tile_pool(name="io", bufs=6) as io,
            tc.tile_pool(name="wk", bufs=4) as wk,
            tc.tile_pool(name="outp", bufs=4) as outp,
            tc.tile_pool(name="sm", bufs=1) as sm,
        ):
            xb = sm.tile([P, B], BF16, tag="xb")
            nc.scalar.dma_start(out=xb[:], in_=xb_d[:, :])
            fvg = sm.tile([P, 3 * NT], F32, tag="fvg")
            nc.scalar.dma_start(out=fvg[:], in_=fvg_d[:, :])
            fn = fvg[:, 0:NT]
            va = fvg[:, NT:2 * NT]
            sg = fvg[:, 2 * NT:3 * NT]
            for T in range(NT):
                r0 = P * T
                ti = io.tile([P, 2 * B], BF16, tag="sv")
                nc.sync.dma_start(out=ti[:], in_=sv_d[r0:r0 + P, :])
                s_t, v_t = ti[:, 0:B], ti[:, B:2 * B]
                d1 = wk.tile([P, B], BF16, tag="d1")
                nc.vector.tensor_tensor(d1[:], s_t, xb[:], OP.subtract)
                m1 = wk.tile([P, B], BF16, tag="m1")
                nc.scalar.mul(m1[:], d1[:], fn[:, T:T + 1])
                q2 = wk.tile([P, B], BF16, tag="q2")
                nc.scalar.activation(q2[:], d1[:], AF.Square,
                                     scale=sg[:, T:T + 1])
                av = wk.tile([P, B], BF16, tag="av")
                nc.vector.tensor_scalar(out=av[:], in0=v_t,
                                        scalar1=va[:, T:T + 1], scalar2=None,
                                        op0=OP.mult)
                ot = outp.tile([P, 2 * B], BF16, tag="ot")
                nc.vector.tensor_tensor(ot[:, 0:B], m1[:], s_t, OP.add)
                nc.vector.tensor_tensor(ot[:, B:2 * B], q2[:], av[:], OP.add)
                nc.gpsimd.dma_start(out=out_d[r0:r0 + P, :], in_=ot[:])
    nc.finalize()
    return nc


_CACHE = {}


def _get(name, builder, *args):
    if name not in _CACHE:
        _CACHE[name] = builder(*args)
    return _CACHE[name]


def _unit_major(a):
    """[S, S] -> [9216, 784]: one 28x28 block per row."""
    return (np.ascontiguousarray(a).reshape(N, IMG, N, IMG)
            .transpose(0, 2, 1, 3).reshape(UNITS, B))


def kernel(som, running_variance, learning_rates, radius,
           cartesian_distances, x):
    som = np.asarray(som, np.float32)
    rv = np.asarray(running_variance, np.float32)
    lr = np.asarray(learning_rates, np.float32)
    rad = np.asarray(radius, np.float32)
    cd = np.asarray(cartesian_distances, np.float32)
    x = np.asarray(x, np.float32)

    som_t = _unit_major(som)
    som_b = som_t.astype(NPBF)
    xb = np.broadcast_to(x.reshape(1, B), (P, B)).astype(NPBF)
    xb = np.ascontiguousarray(xb)

    uniform = bool((rv == rv.flat[0]).all()) and rv.flat[0] > 0
    if uniform:
        rvv = np.full((P, 1), rv.flat[0], np.float32)
        l1_maps = [{
            "s": np.ascontiguousarray(som_b[UPC * c:UPC * (c + 1)]),
            "xb": xb,
            "rvv": rvv,
        } for c in range(NCORES)]
        res1 = bass_utils.run_bass_kernel_spmd(
            _get("l1pu2", _build_l1pu2), l1_maps,
            core_ids=list(range(NCORES)))

        def rv_rows(rows):
            return np.full((len(rows), B), rv.flat[0], NPBF)
    else:
        rv_t = _unit_major(rv)
        rv_b = rv_t.astype(NPBF)
        isr_b = (np.float32(1.0) / np.sqrt(rv_t)).astype(NPBF)
        sv_full = np.concatenate([som_b, rv_b, isr_b], axis=1)
        l1_maps = [{
            "sv": np.ascontiguousarray(sv_full[UPC * c:UPC * (c + 1)]),
            "xb": xb,
        } for c in range(NCORES)]
        res1 = bass_utils.run_bass_kernel_spmd(
            _get("l1pg", _build_l1p, True), l1_maps,
            core_ids=list(range(NCORES)))

        def rv_rows(rows):
            return rv_b[rows]

    # passthrough output (exact for every out-of-radius unit) + um partials
    out_t = np.concatenate(
        [np.asarray(res1.results[c]["out_t"]) for c in range(NCORES)], axis=0)
    um = np.concatenate(
        [np.asarray(res1.results[c]["um"], np.float32).T.reshape(-1)
         for c in range(NCORES)])

    # ---- host glue: argmin + neighborhood factor maps (numpy f32) ----
    g = int(np.argmin(um))
    bi, bj = g // N, g % N
    r = rad[bi, bj]
    lr_s = lr[bi, bj]
    dist_mod = np.float32(1.0) / (np.float32(2.0) * r * r)
    constant = -np.log(EPS_LOG / lr_s) / dist_mod
    d = cd[:, :, bi, bj]
    mask = np.where(d > r, np.float32(0.0), np.float32(1.0))
    fm = mask * lr * np.exp(-d * dist_mod)
    va = RV_ALPHA - np.float32(0.5) + np.float32(1.0) / (
        np.float32(1.0) + np.exp(-d / constant))
    va = np.clip(va * mask + (np.float32(1.0) - mask),
                 np.float32(0.0), np.float32(1.0))
    fn_m = (-fm).reshape(-1).astype(np.float32)
    va_m = va.reshape(-1).astype(np.float32)
    sg_m = np.sqrt((np.float32(1.0) - va) * (np.float32(1.0) - fm) ** 2
                   ).reshape(-1).astype(np.float32)

    idx = np.flatnonzero(mask.reshape(-1) > 0)
    if idx.size == 0:
        pass  # empty neighborhood: passthrough IS the full update
    elif idx.size <= FIX_CAP:
        # ---- sparse fixup: <=1024 affected rows, 128 per core ----
        k = idx.size
        idx_pad = np.concatenate(
            [idx, np.full(FIX_CAP - k, idx[0], np.int64)])
        fvg = np.zeros((FIX_CAP, 3), np.float32)
        fvg[:k, 0] = fn_m[idx]
        fvg[:k, 2] = sg_m[idx]
        if uniform:
            # va*rv folds into a per-partition constant; som rows only
            fvg[:k, 1] = va_m[idx] * np.float32(rv.flat[0])
            fvg[k:, 1] = np.float32(rv.flat[0])
            l2_maps = [{
                "sx": np.ascontiguousarray(np.concatenate(
                    [som_b[idx_pad[P * c:P * (c + 1)]], xb], axis=1)),
                "fvg": np.ascontiguousarray(fvg[P * c:P * (c + 1)]),
            } for c in range(NCORES)]
            nc2 = _get("l2su", _build_l2su)
        else:
            fvg[:k, 1] = va_m[idx]
            fvg[k:, 1] = 1.0    # padding rows: identity update
            svx = np.concatenate([som_b[idx_pad], rv_rows(idx_pad)], axis=1)
            l2_maps = [{
                "svx": np.ascontiguousarray(svx[P * c:P * (c + 1)]),
                "xb": xb,
                "fvg": np.ascontiguousarray(fvg[P * c:P * (c + 1)]),
            } for c in range(NCORES)]
            nc2 = _get("l2s", _build_l2s)
        res2 = bass_utils.run_bass_kernel_spmd(
            nc2, l2_maps, core_ids=list(range(NCORES)))
        fix = np.concatenate(
            [np.asarray(res2.results[c]["outx"]) for c in range(NCORES)],
            axis=0)
        out_t[idx] = fix[:k]
    else:
        # ---- dense fallback: update every unit ----
        def shard(vec, c):
            return vec[UPC * c:UPC * (c + 1)].reshape(NT, P).T.copy()
        all_rows = np.arange(UNITS)
        sv2 = np.concatenate([som_b, rv_rows(all_rows)], axis=1)
        l2_maps = [{
            "sv": np.ascontiguousarray(sv2[UPC * c:UPC * (c + 1)]),
            "xb": xb,
            "fvg": np.ascontiguousarray(np.concatenate(
                [shard(fn_m, c), shard(va_m, c), shard(sg_m, c)], axis=1)),
        } for c in range(NCORES)]
        res2 = bass_utils.run_bass_kernel_spmd(
            _get("l2d", _build_l2d), l2_maps, core_ids=list(range(NCORES)))
        out_t = np.concatenate(
            [np.asarray(res2.results[c]["out_t"]) for c in range(NCORES)],
            axis=0)

    out_t = out_t.astype(np.float32)
    sn_t, vn_t = out_t[:, 0:B], out_t[:, B:2 * B]

    def untile(a):
        return (a.reshape(N, N, IMG, IMG).transpose(0, 2, 1, 3)
                .reshape(S, S))

    return np.stack([untile(sn_t), untile(vn_t)]).astype(np.float32)
